# revision 52
# baseline (speedup 1.0000x reference)
"""AdaptiveLiquidNeuron forward on 8 TRN2 NeuronCores (data-parallel over batch).

Math (per batch row, H=1024):
  context = relu(h @ W1.T + b1) @ W2.T + b2
  pa      = context @ PM.T + pm_b
  mm      = (1 + pa) * (e @ Wrec.T)
  dh      = (-decay*h + mm + bias) / (tau * sigmoid(pa))
  out     = LayerNorm(dh) * ln_w + ln_b

Strategy: shard B=16384 over 8 cores (2048 rows each), replicate H x H weights.
On-chip everything is kept transposed ([H on partitions, B on free]) so the four
matmuls need no on-chip transposes (host pre-transposes weights + activations).
LayerNorm reduces over the partition axis via ones-matmuls (sum and sum-of-
squares side by side in one rhs); rstd = 2*Dsqrt(var+eps) with the 2 folded into
ln_w host-side; stats broadcast back across partitions with one K=1 matmul.
Host folds 1/tau into Wrec/decay/bias, ce_b2 into pm_b, and uses
1/sigmoid(x) = 1 + exp(-x).
"""

import numpy as np
import ml_dtypes

BF16 = ml_dtypes.bfloat16

B, H = 16384, 1024
NCORES = 8
BL = B // NCORES      # 2048 batch rows per core
P = 128               # partitions
KC = H // P           # 8 chunks of the hidden dim
NB = 8                # batch tiles per core
NT = BL // NB         # 256 batch columns per tile
EPS = 1e-5

# consts layout: [128, 6*KC] f32, column v*KC + m = chunk m of vector v
V_B1, V_PMB, V_NDEC, V_BIASP, V_LNW2, V_LNB = range(6)

_CACHED = {}


def _build_nc(lnb_zero):
    import concourse.bacc as bacc
    import concourse.tile as tile
    from concourse import mybir
    from contextlib import ExitStack

    f32 = mybir.dt.float32
    bf16 = mybir.dt.bfloat16
    AF = mybir.ActivationFunctionType
    OP = mybir.AluOpType

    nc = bacc.Bacc(target_bir_lowering=False)

    # h/e/out are tile-major on DRAM ([NB, H, NT]) so every DMA is contiguous
    hT_e = nc.declare_dram_parameter("hT", [NB, H, NT], bf16, isOutput=False)
    eT_e = nc.declare_dram_parameter("eT", [NB, H, NT], bf16, isOutput=False)
    w1_e = nc.declare_dram_parameter("w1T", [H, H], bf16, isOutput=False)
    w2_e = nc.declare_dram_parameter("w2T", [H, H], bf16, isOutput=False)
    pm_e = nc.declare_dram_parameter("pmT", [H, H], bf16, isOutput=False)
    wr_e = nc.declare_dram_parameter("wrT", [H, H], bf16, isOutput=False)
    cs_e = nc.declare_dram_parameter("consts", [P, 6 * KC], f32, isOutput=False)
    out_e = nc.declare_dram_parameter("out", [NB, H, NT], f32, isOutput=True)

    hT_r = hT_e[:].rearrange("i (k p) b -> i p k b", p=P)
    eT_r = eT_e[:].rearrange("i (k p) b -> i p k b", p=P)
    out_r = out_e[:].rearrange("i (m p) b -> i p m b", p=P)

    with tile.TileContext(nc) as tc, ExitStack() as ctx:
        wpool = ctx.enter_context(tc.tile_pool(name="weights", bufs=1))
        cpool = ctx.enter_context(tc.tile_pool(name="consts", bufs=1))
        iopool = ctx.enter_context(tc.tile_pool(name="io", bufs=3))
        actpool = ctx.enter_context(tc.tile_pool(name="acts", bufs=1))
        fpool = ctx.enter_context(tc.tile_pool(name="f32work", bufs=1))
        dhpool = ctx.enter_context(tc.tile_pool(name="dh", bufs=2))
        rpool = ctx.enter_context(tc.tile_pool(name="redu", bufs=2))
        ypool = ctx.enter_context(tc.tile_pool(name="y", bufs=4))
        rowpool = ctx.enter_context(tc.tile_pool(name="rows", bufs=2))
        outpool = ctx.enter_context(tc.tile_pool(name="outs", bufs=1))
        psA = ctx.enter_context(tc.tile_pool(name="psA", bufs=4, space="PSUM"))
        psS = ctx.enter_context(tc.tile_pool(name="psS", bufs=1, space="PSUM"))
        psB = ctx.enter_context(tc.tile_pool(name="psB", bufs=2, space="PSUM"))

        # ---- resident constants / weights ----
        # Prologue latency: mm1 needs consts+w1+hT0 first — split those
        # halves across the sync and gpsimd DMA queues so they stream in
        # parallel; everything else queues up behind in need-order.
        consts = cpool.tile([P, 6 * KC], f32, tag="consts")
        nc.sync.dma_start(out=consts[:], in_=cs_e[:])

        def col(v, m):
            return consts[:, v * KC + m : v * KC + m + 1]

        w_sb = {}
        for nm, ext in (("w1", w1_e), ("w2", w2_e), ("pm", pm_e), ("wr", wr_e)):
            w_sb[nm] = (wpool.tile([P, KC, H], bf16, tag=nm, name=f"w_{nm}"), ext)

        def load_w(nm, eng, lo=0, hi=KC):
            t, ext = w_sb[nm]
            src = ext[:].rearrange("(k p) m -> p k m", p=P)
            eng.dma_start(out=t[:, lo:hi, :], in_=src[:, lo:hi, :])
            return t

        def load_io(i, h_eng, e_eng, split=False):
            ht = iopool.tile([P, KC, NT], bf16, tag="hT")
            et = iopool.tile([P, KC, NT], bf16, tag="eT")
            if split:
                h_eng.dma_start(out=ht[:, 0 : KC // 2, :],
                                in_=hT_r[i, :, 0 : KC // 2, :])
                e_eng.dma_start(out=ht[:, KC // 2 :, :],
                                in_=hT_r[i, :, KC // 2 :, :])
            else:
                h_eng.dma_start(out=ht[:], in_=hT_r[i])
            e_eng.dma_start(out=et[:], in_=eT_r[i])
            return ht, et

        # Prologue: ~1MB pieces (descriptor-efficient) spread over the two
        # HWDGE rings (SP, ACT) + SWDGE (gpsimd) in the order compute needs
        # them: w1+h0 (mm1), w2 (mm2), pm (mm3), wr+e0 (mm4), then tile 1.
        ht0 = iopool.tile([P, KC, NT], bf16, tag="hT")
        et0 = iopool.tile([P, KC, NT], bf16, tag="eT")
        w1_sb = load_w("w1", nc.sync, 0, 4)
        nc.scalar.dma_start(out=ht0[:, 0:4, :], in_=hT_r[0, :, 0:4, :])
        load_w("w1", nc.sync, 4, 8)
        nc.scalar.dma_start(out=ht0[:, 4:8, :], in_=hT_r[0, :, 4:8, :])
        w2_sb = load_w("w2", nc.scalar, 0, 4)
        load_w("w2", nc.gpsimd, 4, 8)
        pm_sb = load_w("pm", nc.scalar, 0, 4)
        load_w("pm", nc.gpsimd, 4, 8)
        wr_sb = load_w("wr", nc.sync, 0, 4)
        load_w("wr", nc.scalar, 4, 8)
        nc.gpsimd.dma_start(out=et0[:], in_=eT_r[0])
        io_tiles = [(ht0, et0), None]
        io_tiles[1] = load_io(1, nc.gpsimd, nc.gpsimd)

        ones_col = cpool.tile([P, 1], bf16, tag="ones_col")
        nc.vector.memset(ones_col[:], 1.0)
        ones_row = cpool.tile([1, P], f32, tag="ones_row")
        nc.vector.memset(ones_row[:], 1.0)


        state = [None] * NB

        def mm_layer(w, rhs_t, evac):
            """psum[m] = w[:,:,m].T @ rhs (contract KC chunks); evac(m, psum)."""
            for m in range(KC):
                acc = psA.tile([P, NT], f32, tag="acc")
                for k in range(KC):
                    nc.tensor.matmul(
                        acc[:],
                        w[:, k, m * P : (m + 1) * P],
                        rhs_t[:, k, :],
                        start=(k == 0),
                        stop=(k == KC - 1),
                    )
                evac(m, acc)

        def matmul_phase(i, pe_hook1, pe_hook2):
            ht, et = io_tiles[i % 2]
            if i + 2 < NB:
                io_tiles[i % 2] = load_io(i + 2, nc.sync, nc.sync)

            c1 = actpool.tile([P, KC, NT], bf16, tag="c1")
            cx = actpool.tile([P, KC, NT], bf16, tag="ctx")
            pa = fpool.tile([P, KC, NT], f32, tag="pa")
            ex = fpool.tile([P, KC, NT], f32, tag="exp")
            t2 = fpool.tile([P, KC, NT], f32, tag="t2")
            u = fpool.tile([P, KC, NT], f32, tag="u")
            num = fpool.tile([P, KC, NT], f32, tag="num")
            dh = dhpool.tile([P, KC, NT], f32, tag="dh")
            # dh (bf16) and dh^2 side by side so one ones-matmul per chunk
            # yields both sum and sum-of-squares
            red = rpool.tile([P, KC, 2 * NT], bf16, tag="red")

            # u only needs hT + consts: emit first so it clears the in-order
            # gpsimd queue before tile i-1's row math lands there
            for m in range(KC):
                nc.gpsimd.tensor_scalar(
                    u[:, m, :],
                    ht[:, m, :],
                    col(V_NDEC, m),
                    col(V_BIASP, m),
                    op0=OP.mult,
                    op1=OP.add,
                )

            # context encoder layer 1: c1 = relu(W1 @ hT + b1)
            def relu_evac(m, acc):
                nc.scalar.activation(
                    c1[:, m, :], acc[:], AF.Relu, bias=col(V_B1, m), scale=1.0
                )

            if i == 0:
                # k-outer in m-halves: consumes w1/hT chunks as the DMAs
                # land instead of waiting for the full tensors
                for half in range(2):
                    ms_ = range(half * 4, half * 4 + 4)
                    accs = [
                        psA.tile([P, NT], f32, tag="acc", name=f"acc0_{m}")
                        for m in ms_
                    ]
                    for k in range(KC):
                        for j, m in enumerate(ms_):
                            nc.tensor.matmul(
                                accs[j][:],
                                w1_sb[:, k, m * P : (m + 1) * P],
                                ht[:, k, :],
                                start=(k == 0),
                                stop=(k == KC - 1),
                            )
                    for j, m in enumerate(ms_):
                        relu_evac(m, accs[j])
            else:
                mm_layer(w1_sb, ht, relu_evac)
            pe_hook1()  # reductions of tile i-1 slot in here on PE
            # context encoder layer 2 (b2 folded into pm_b): ctx = W2 @ c1
            mm_layer(
                w2_sb,
                c1,
                lambda m, acc: nc.scalar.activation(
                    cx[:, m, :], acc[:], AF.Copy, bias=0.0, scale=1.0
                ),
            )
            pe_hook2()  # stat broadcast of tile i-1
            # param modulator: pa = PM @ ctx + pm_b'
            mm_layer(
                pm_sb,
                cx,
                lambda m, acc: nc.vector.tensor_scalar_add(
                    pa[:, m, :], acc[:], col(V_PMB, m)
                ),
            )
            # 1/sigmoid(pa) = 1 + exp(-pa)
            nc.scalar.activation(ex[:], pa[:], AF.Exp, bias=0.0, scale=-1.0)

            # recurrent: t2 = (1 + pa) * (Wrec' @ eT)
            mm_layer(
                wr_sb,
                et,
                lambda m, acc: nc.vector.scalar_tensor_tensor(
                    t2[:, m, :], pa[:, m, :], 1.0, acc[:], op0=OP.add, op1=OP.mult
                ),
            )

            # halves keep the tail latency down: reduce matmuls for half 0
            # can start while half 1 is still in the vector pipe
            nsplit = 4 if i == NB - 1 else 2  # short tail for the last tile
            step = KC // nsplit
            for s in [slice(j * step, (j + 1) * step) for j in range(nsplit)]:
                nc.vector.tensor_add(num[:, s, :], t2[:, s, :], u[:, s, :])
                # dh = num * (1 + exp(-pa))
                nc.vector.scalar_tensor_tensor(
                    dh[:, s, :], ex[:, s, :], 1.0, num[:, s, :],
                    op0=OP.add, op1=OP.mult,
                )
                nc.scalar.square(red[:, s, NT:], dh[:, s, :])
                nc.scalar.copy(red[:, s, 0:NT], dh[:, s, :])
            state[i] = (dh, red)

        def reduce_phase(i):
            # partition-axis sum+sumsq via ones-matmuls over all H=1024
            dh, red = state[i]
            s_ps = psS.tile([1, 2 * NT], f32, tag="sums")
            for m in range(KC):
                nc.tensor.matmul(
                    s_ps[:], ones_col[:], red[:, m, :],
                    start=(m == 0), stop=(m == KC - 1),
                )
            i32 = mybir.dt.int32
            mu_n = rowpool.tile([1, NT], f32, tag="mu_n")
            ms = rowpool.tile([1, NT], f32, tag="ms")
            musq = rowpool.tile([1, NT], f32, tag="musq")
            ve = rowpool.tile([1, NT], f32, tag="ve")
            yb = rowpool.tile([1, NT], f32, tag="yb")
            t1 = rowpool.tile([1, NT], f32, tag="t1")
            t2r = rowpool.tile([1, NT], f32, tag="t2r")
            dq = rowpool.tile([1, 2 * NT], f32, tag="dq")
            # rstd = rsqrt(var+eps) via Quake initial guess + one Newton
            # step (rel err ~2e-3, far below bf16 matmul noise) — avoids
            # ln/sqrt ACT funcs so the whole kernel stays in one
            # activation-table set (no table reloads)
            nc.vector.tensor_scalar_mul(mu_n[:], s_ps[:, 0:NT], -1.0 / H)
            nc.vector.tensor_scalar(ms[:], s_ps[:, NT:], 1.0 / H, EPS,
                                    op0=OP.mult, op1=OP.add)
            nc.vector.tensor_mul(musq[:], mu_n[:], mu_n[:])
            nc.vector.tensor_sub(ve[:], ms[:], musq[:])  # var + eps
            nc.vector.tensor_scalar(
                t1[:].bitcast(i32), ve[:].bitcast(i32), 1, None,
                op0=OP.arith_shift_right,
            )
            nc.vector.tensor_scalar(
                yb[:].bitcast(i32), t1[:].bitcast(i32), -1, 0x5F3759DF,
                op0=OP.mult, op1=OP.add,
            )
            # y1 = y0*(1.5 - 0.5*ve*y0^2)
            nc.vector.tensor_mul(t1[:], yb[:], yb[:])
            nc.vector.tensor_mul(t2r[:], t1[:], ve[:])
            nc.vector.tensor_scalar(t2r[:], t2r[:], -0.5, 1.5,
                                    op0=OP.mult, op1=OP.add)
            nc.vector.tensor_mul(dq[:, 0:NT], yb[:], t2r[:])
            nc.vector.tensor_mul(dq[:, NT:], mu_n[:], dq[:, 0:NT])
            state[i] = (dh, dq)

        def bcast_phase(i):
            dh, dq = state[i]
            pq = psB.tile([P, 2 * NT], f32, tag="PQ")
            nc.tensor.matmul(pq[:], ones_row[:], dq[:], start=True, stop=True)
            state[i] = (dh, pq)

        def epilogue_phase(i):
            dh, pq = state[i]
            outf = outpool.tile([P, KC, NT], f32, tag="outf")
            for m in range(KC):
                # out = lnw2*(dh*D + qn) (+ lnb) = ln_w*rstd*(dh-mu) + ln_b
                s1 = ypool.tile([P, NT], f32, tag="s1")
                nc.vector.scalar_tensor_tensor(
                    s1[:], dh[:, m, :], col(V_LNW2, m), pq[:, 0:NT],
                    op0=OP.mult, op1=OP.mult,
                )
                if lnb_zero:
                    nc.vector.scalar_tensor_tensor(
                        outf[:, m, :], pq[:, NT:], col(V_LNW2, m), s1[:],
                        op0=OP.mult, op1=OP.add,
                    )
                else:
                    s2 = ypool.tile([P, NT], f32, tag="s2")
                    nc.vector.scalar_tensor_tensor(
                        s2[:], pq[:, NT:], col(V_LNW2, m), s1[:],
                        op0=OP.mult, op1=OP.add,
                    )
                    nc.vector.tensor_scalar_add(outf[:, m, :], s2[:], col(V_LNB, m))
                if m % 2 == 1:  # stream results out as they complete
                    nc.sync.dma_start(
                        out=out_r[i, :, m - 1 : m + 1, :],
                        in_=outf[:, m - 1 : m + 1, :],
                    )
            state[i] = None

        for i in range(NB):
            matmul_phase(
                i,
                (lambda j=i: reduce_phase(j - 1)) if i > 0 else (lambda: None),
                (lambda j=i: bcast_phase(j - 1)) if i > 0 else (lambda: None),
            )
            if i > 0:
                epilogue_phase(i - 1)
        reduce_phase(NB - 1)
        bcast_phase(NB - 1)
        epilogue_phase(NB - 1)

    if not nc.is_finalized():
        nc.finalize()
    return nc


def _get_nc(lnb_zero):
    key = ("nc", lnb_zero)
    if key not in _CACHED:
        _CACHED[key] = _build_nc(lnb_zero)
    return _CACHED[key]


# test.py can flip these before calling kernel() to profile
TRACE = False
LAST_RESULT = {}


def kernel(t, h, e, W_rec, bias, tau, decay, ln_w, ln_b,
           ce_w1, ce_b1, ce_w2, ce_b2, pm_w, pm_b):
    from concourse.bass_utils import run_bass_kernel_spmd

    f = np.float32
    h = np.asarray(h, f)
    e = np.asarray(e, f)
    W_rec = np.asarray(W_rec, f)
    bias = np.asarray(bias, f)
    tau = np.asarray(tau, f)
    decay = np.asarray(decay, f)
    ln_w = np.asarray(ln_w, f)
    ln_b = np.asarray(ln_b, f)
    ce_w1 = np.asarray(ce_w1, f)
    ce_b1 = np.asarray(ce_b1, f)
    ce_w2 = np.asarray(ce_w2, f)
    ce_b2 = np.asarray(ce_b2, f)
    pm_w = np.asarray(pm_w, f)
    pm_b = np.asarray(pm_b, f)

    invtau = 1.0 / tau
    negdecay = -decay * invtau
    biasp = bias * invtau
    pmb_eff = pm_b + pm_w @ ce_b2  # fold ce_b2 through the param modulator
    lnb_zero = bool(np.all(ln_b == 0.0))

    w1T = np.ascontiguousarray(ce_w1.T).astype(BF16)
    w2T = np.ascontiguousarray(ce_w2.T).astype(BF16)
    pmT = np.ascontiguousarray(pm_w.T).astype(BF16)
    wrT = np.ascontiguousarray(W_rec.T * invtau[None, :]).astype(BF16)

    def chunked(v):  # [H] -> [128, KC] with column m = chunk m
        return np.ascontiguousarray(v.reshape(KC, P).T)

    consts = np.concatenate(
        [chunked(v) for v in (ce_b1, pmb_eff, negdecay, biasp, ln_w, ln_b)],
        axis=1,
    ).astype(f)

    def tile_major(x, rows):  # [BL, H] slice -> [NB, H, NT] transposed tiles
        return np.ascontiguousarray(
            x[rows].reshape(NB, NT, H).transpose(0, 2, 1)
        ).astype(BF16)

    in_maps = []
    for i in range(NCORES):
        rows = slice(i * BL, (i + 1) * BL)
        in_maps.append({
            "hT": tile_major(h, rows),
            "eT": tile_major(e, rows),
            "w1T": w1T, "w2T": w2T, "pmT": pmT, "wrT": wrT,
            "consts": consts,
        })

    nc = _get_nc(lnb_zero)
    res = run_bass_kernel_spmd(nc, in_maps, core_ids=list(range(NCORES)),
                               trace=TRACE)
    LAST_RESULT["exec_time_ns"] = res.exec_time_ns
    LAST_RESULT["mean_exec_time_ns"] = res.mean_exec_time_ns
    LAST_RESULT["instructions_and_trace"] = res.instructions_and_trace

    out = np.empty((B, H), f)
    for i in range(NCORES):
        # [NB, H, NT] tile-major transposed -> [BL, H]
        out[i * BL : (i + 1) * BL] = (
            res.results[i]["out"].transpose(0, 2, 1).reshape(BL, H)
        )
    return out


# revision 54
# speedup vs baseline: 1.0347x; 1.0347x over previous
"""AdaptiveLiquidNeuron forward on 8 TRN2 NeuronCores (data-parallel over batch).

Math (per batch row, H=1024):
  context = relu(h @ W1.T + b1) @ W2.T + b2
  pa      = context @ PM.T + pm_b
  mm      = (1 + pa) * (e @ Wrec.T)
  dh      = (-decay*h + mm + bias) / (tau * sigmoid(pa))
  out     = LayerNorm(dh) * ln_w + ln_b

Strategy: shard B=16384 over 8 cores (2048 rows each), replicate H x H weights.
On-chip everything is kept transposed ([H on partitions, B on free]) so the four
matmuls need no on-chip transposes (host pre-transposes weights + activations).
LayerNorm reduces over the partition axis via ones-matmuls (sum and sum-of-
squares side by side in one rhs); rstd = 2*Dsqrt(var+eps) with the 2 folded into
ln_w host-side; stats broadcast back across partitions with one K=1 matmul.
Host folds 1/tau into Wrec/decay/bias, ce_b2 into pm_b, and uses
1/sigmoid(x) = 1 + exp(-x).
"""

import numpy as np
import ml_dtypes

BF16 = ml_dtypes.bfloat16

B, H = 16384, 1024
NCORES = 8
BL = B // NCORES      # 2048 batch rows per core
P = 128               # partitions
KC = H // P           # 8 chunks of the hidden dim
NB = 8                # batch tiles per core
NT = BL // NB         # 256 batch columns per tile
EPS = 1e-5

# consts layout: [128, 6*KC] f32, column v*KC + m = chunk m of vector v
V_B1, V_PMB, V_NDEC, V_BIASP, V_LNW2, V_LNB = range(6)

_CACHED = {}


def _build_nc(lnb_zero):
    import concourse.bacc as bacc
    import concourse.tile as tile
    from concourse import mybir
    from contextlib import ExitStack

    f32 = mybir.dt.float32
    bf16 = mybir.dt.bfloat16
    AF = mybir.ActivationFunctionType
    OP = mybir.AluOpType

    nc = bacc.Bacc(target_bir_lowering=False)

    # h/e/out are tile-major on DRAM ([NB, H, NT]) so every DMA is contiguous
    hT_e = nc.declare_dram_parameter("hT", [NB, H, NT], bf16, isOutput=False)
    eT_e = nc.declare_dram_parameter("eT", [NB, H, NT], bf16, isOutput=False)
    w1_e = nc.declare_dram_parameter("w1T", [H, H], bf16, isOutput=False)
    w2_e = nc.declare_dram_parameter("w2T", [H, H], bf16, isOutput=False)
    pm_e = nc.declare_dram_parameter("pmT", [H, H], bf16, isOutput=False)
    wr_e = nc.declare_dram_parameter("wrT", [H, H], bf16, isOutput=False)
    cs_e = nc.declare_dram_parameter("consts", [P, 6 * KC], f32, isOutput=False)
    out_e = nc.declare_dram_parameter("out", [NB, H, NT], f32, isOutput=True)

    hT_r = hT_e[:].rearrange("i (k p) b -> i p k b", p=P)
    eT_r = eT_e[:].rearrange("i (k p) b -> i p k b", p=P)
    out_r = out_e[:].rearrange("i (m p) b -> i p m b", p=P)

    with tile.TileContext(nc) as tc, ExitStack() as ctx:
        wpool = ctx.enter_context(tc.tile_pool(name="weights", bufs=1))
        cpool = ctx.enter_context(tc.tile_pool(name="consts", bufs=1))
        iopool = ctx.enter_context(tc.tile_pool(name="io", bufs=3))
        actpool = ctx.enter_context(tc.tile_pool(name="acts", bufs=1))
        fpool = ctx.enter_context(tc.tile_pool(name="f32work", bufs=1))
        dhpool = ctx.enter_context(tc.tile_pool(name="dh", bufs=2))
        rpool = ctx.enter_context(tc.tile_pool(name="redu", bufs=2))
        ypool = ctx.enter_context(tc.tile_pool(name="y", bufs=4))
        rowpool = ctx.enter_context(tc.tile_pool(name="rows", bufs=2))
        outpool = ctx.enter_context(tc.tile_pool(name="outs", bufs=1))
        psA = ctx.enter_context(tc.tile_pool(name="psA", bufs=4, space="PSUM"))
        psS = ctx.enter_context(tc.tile_pool(name="psS", bufs=1, space="PSUM"))
        psB = ctx.enter_context(tc.tile_pool(name="psB", bufs=2, space="PSUM"))

        # ---- resident constants / weights ----
        # Prologue latency: mm1 needs consts+w1+hT0 first — split those
        # halves across the sync and gpsimd DMA queues so they stream in
        # parallel; everything else queues up behind in need-order.
        consts = cpool.tile([P, 6 * KC], f32, tag="consts")
        nc.gpsimd.dma_start(out=consts[:], in_=cs_e[:])

        def col(v, m):
            return consts[:, v * KC + m : v * KC + m + 1]

        w_sb = {}
        for nm, ext in (("w1", w1_e), ("w2", w2_e), ("pm", pm_e), ("wr", wr_e)):
            w_sb[nm] = (wpool.tile([P, KC, H], bf16, tag=nm, name=f"w_{nm}"), ext)

        def load_w(nm, eng, lo=0, hi=KC):
            t, ext = w_sb[nm]
            src = ext[:].rearrange("(k p) m -> p k m", p=P)
            eng.dma_start(out=t[:, lo:hi, :], in_=src[:, lo:hi, :])
            return t

        def load_io(i, h_eng, e_eng, split=False):
            ht = iopool.tile([P, KC, NT], bf16, tag="hT")
            et = iopool.tile([P, KC, NT], bf16, tag="eT")
            if split:
                h_eng.dma_start(out=ht[:, 0 : KC // 2, :],
                                in_=hT_r[i, :, 0 : KC // 2, :])
                e_eng.dma_start(out=ht[:, KC // 2 :, :],
                                in_=hT_r[i, :, KC // 2 :, :])
            else:
                h_eng.dma_start(out=ht[:], in_=hT_r[i])
            e_eng.dma_start(out=et[:], in_=eT_r[i])
            return ht, et

        # Prologue: ~1MB pieces (descriptor-efficient) spread over the two
        # HWDGE rings (SP, ACT) + SWDGE (gpsimd) in the order compute needs
        # them: w1+h0 (mm1), w2 (mm2), pm (mm3), wr+e0 (mm4), then tile 1.
        ht0 = iopool.tile([P, KC, NT], bf16, tag="hT")
        et0 = iopool.tile([P, KC, NT], bf16, tag="eT")
        w1_sb = load_w("w1", nc.sync, 0, 4)
        nc.scalar.dma_start(out=ht0[:, 0:4, :], in_=hT_r[0, :, 0:4, :])
        load_w("w1", nc.scalar, 4, 8)
        nc.sync.dma_start(out=ht0[:, 4:8, :], in_=hT_r[0, :, 4:8, :])
        w2_sb = load_w("w2", nc.sync, 0, 4)
        load_w("w2", nc.scalar, 4, 8)
        pm_sb = load_w("pm", nc.sync, 0, 4)
        load_w("pm", nc.scalar, 4, 8)
        wr_sb = load_w("wr", nc.sync, 0, 4)
        load_w("wr", nc.scalar, 4, 8)
        nc.gpsimd.dma_start(out=et0[:], in_=eT_r[0])
        io_tiles = [(ht0, et0), None]
        io_tiles[1] = load_io(1, nc.gpsimd, nc.gpsimd)

        ones_col = cpool.tile([P, 1], bf16, tag="ones_col")
        nc.vector.memset(ones_col[:], 1.0)
        ones_row = cpool.tile([1, P], f32, tag="ones_row")
        nc.vector.memset(ones_row[:], 1.0)


        state = [None] * NB

        def mm_layer(w, rhs_t, evac):
            """psum[m] = w[:,:,m].T @ rhs (contract KC chunks); evac(m, psum)."""
            for m in range(KC):
                acc = psA.tile([P, NT], f32, tag="acc")
                for k in range(KC):
                    nc.tensor.matmul(
                        acc[:],
                        w[:, k, m * P : (m + 1) * P],
                        rhs_t[:, k, :],
                        start=(k == 0),
                        stop=(k == KC - 1),
                    )
                evac(m, acc)

        def matmul_phase(i, pe_hook1, pe_hook2):
            ht, et = io_tiles[i % 2]
            if i + 2 < NB:
                io_tiles[i % 2] = load_io(i + 2, nc.sync, nc.sync)

            c1 = actpool.tile([P, KC, NT], bf16, tag="c1")
            cx = actpool.tile([P, KC, NT], bf16, tag="ctx")
            pa = fpool.tile([P, KC, NT], f32, tag="pa")
            ex = fpool.tile([P, KC, NT], f32, tag="exp")
            t2 = fpool.tile([P, KC, NT], f32, tag="t2")
            u = fpool.tile([P, KC, NT], f32, tag="u")
            num = fpool.tile([P, KC, NT], f32, tag="num")
            dh = dhpool.tile([P, KC, NT], f32, tag="dh")
            # dh (bf16) and dh^2 side by side so one ones-matmul per chunk
            # yields both sum and sum-of-squares
            red = rpool.tile([P, KC, 2 * NT], bf16, tag="red")

            # u only needs hT + consts: emit first so it clears the in-order
            # gpsimd queue before tile i-1's row math lands there
            for m in range(KC):
                nc.gpsimd.tensor_scalar(
                    u[:, m, :],
                    ht[:, m, :],
                    col(V_NDEC, m),
                    col(V_BIASP, m),
                    op0=OP.mult,
                    op1=OP.add,
                )

            # context encoder layer 1: c1 = relu(W1 @ hT + b1)
            def relu_evac(m, acc):
                nc.scalar.activation(
                    c1[:, m, :], acc[:], AF.Relu, bias=col(V_B1, m), scale=1.0
                )

            if i == 0:
                # k-outer in m-halves: consumes w1/hT chunks as the DMAs
                # land instead of waiting for the full tensors
                for half in range(2):
                    ms_ = range(half * 4, half * 4 + 4)
                    accs = [
                        psA.tile([P, NT], f32, tag="acc", name=f"acc0_{m}")
                        for m in ms_
                    ]
                    for k in range(KC):
                        for j, m in enumerate(ms_):
                            nc.tensor.matmul(
                                accs[j][:],
                                w1_sb[:, k, m * P : (m + 1) * P],
                                ht[:, k, :],
                                start=(k == 0),
                                stop=(k == KC - 1),
                            )
                    for j, m in enumerate(ms_):
                        relu_evac(m, accs[j])
            else:
                mm_layer(w1_sb, ht, relu_evac)
            pe_hook1()  # reductions of tile i-1 slot in here on PE
            # context encoder layer 2 (b2 folded into pm_b): ctx = W2 @ c1
            mm_layer(
                w2_sb,
                c1,
                lambda m, acc: nc.scalar.activation(
                    cx[:, m, :], acc[:], AF.Copy, bias=0.0, scale=1.0
                ),
            )
            pe_hook2()  # stat broadcast of tile i-1
            # param modulator: pa = PM @ ctx + pm_b'
            mm_layer(
                pm_sb,
                cx,
                lambda m, acc: nc.vector.tensor_scalar_add(
                    pa[:, m, :], acc[:], col(V_PMB, m)
                ),
            )
            # 1/sigmoid(pa) = 1 + exp(-pa)
            nc.scalar.activation(ex[:], pa[:], AF.Exp, bias=0.0, scale=-1.0)

            # recurrent: t2 = (1 + pa) * (Wrec' @ eT)
            mm_layer(
                wr_sb,
                et,
                lambda m, acc: nc.vector.scalar_tensor_tensor(
                    t2[:, m, :], pa[:, m, :], 1.0, acc[:], op0=OP.add, op1=OP.mult
                ),
            )

            # halves keep the tail latency down: reduce matmuls for half 0
            # can start while half 1 is still in the vector pipe
            nsplit = 4 if i == NB - 1 else 2  # short tail for the last tile
            step = KC // nsplit
            for s in [slice(j * step, (j + 1) * step) for j in range(nsplit)]:
                nc.vector.tensor_add(num[:, s, :], t2[:, s, :], u[:, s, :])
                # dh = num * (1 + exp(-pa))
                nc.vector.scalar_tensor_tensor(
                    dh[:, s, :], ex[:, s, :], 1.0, num[:, s, :],
                    op0=OP.add, op1=OP.mult,
                )
                nc.scalar.square(red[:, s, NT:], dh[:, s, :])
                nc.scalar.copy(red[:, s, 0:NT], dh[:, s, :])
            state[i] = (dh, red)

        def reduce_phase(i):
            # partition-axis sum+sumsq via ones-matmuls over all H=1024
            dh, red = state[i]
            s_ps = psS.tile([1, 2 * NT], f32, tag="sums")
            for m in range(KC):
                nc.tensor.matmul(
                    s_ps[:], ones_col[:], red[:, m, :],
                    start=(m == 0), stop=(m == KC - 1),
                )
            i32 = mybir.dt.int32
            mu_n = rowpool.tile([1, NT], f32, tag="mu_n")
            ms = rowpool.tile([1, NT], f32, tag="ms")
            musq = rowpool.tile([1, NT], f32, tag="musq")
            ve = rowpool.tile([1, NT], f32, tag="ve")
            yb = rowpool.tile([1, NT], f32, tag="yb")
            t1 = rowpool.tile([1, NT], f32, tag="t1")
            t2r = rowpool.tile([1, NT], f32, tag="t2r")
            dq = rowpool.tile([1, 2 * NT], f32, tag="dq")
            # rstd = rsqrt(var+eps) via Quake initial guess + one Newton
            # step (rel err ~2e-3, far below bf16 matmul noise) — avoids
            # ln/sqrt ACT funcs so the whole kernel stays in one
            # activation-table set (no table reloads)
            nc.vector.tensor_scalar_mul(mu_n[:], s_ps[:, 0:NT], -1.0 / H)
            nc.vector.tensor_scalar(ms[:], s_ps[:, NT:], 1.0 / H, EPS,
                                    op0=OP.mult, op1=OP.add)
            nc.vector.tensor_mul(musq[:], mu_n[:], mu_n[:])
            nc.vector.tensor_sub(ve[:], ms[:], musq[:])  # var + eps
            nc.vector.tensor_scalar(
                t1[:].bitcast(i32), ve[:].bitcast(i32), 1, None,
                op0=OP.arith_shift_right,
            )
            nc.vector.tensor_scalar(
                yb[:].bitcast(i32), t1[:].bitcast(i32), -1, 0x5F3759DF,
                op0=OP.mult, op1=OP.add,
            )
            # y1 = y0*(1.5 - 0.5*ve*y0^2)
            nc.vector.tensor_mul(t1[:], yb[:], yb[:])
            nc.vector.tensor_mul(t2r[:], t1[:], ve[:])
            nc.vector.tensor_scalar(t2r[:], t2r[:], -0.5, 1.5,
                                    op0=OP.mult, op1=OP.add)
            nc.vector.tensor_mul(dq[:, 0:NT], yb[:], t2r[:])
            nc.vector.tensor_mul(dq[:, NT:], mu_n[:], dq[:, 0:NT])
            state[i] = (dh, dq)

        def bcast_phase(i):
            dh, dq = state[i]
            pq = psB.tile([P, 2 * NT], f32, tag="PQ")
            nc.tensor.matmul(pq[:], ones_row[:], dq[:], start=True, stop=True)
            state[i] = (dh, pq)

        def epilogue_phase(i):
            dh, pq = state[i]
            outf = outpool.tile([P, KC, NT], f32, tag="outf")
            for m in range(KC):
                # out = lnw2*(dh*D + qn) (+ lnb) = ln_w*rstd*(dh-mu) + ln_b
                s1 = ypool.tile([P, NT], f32, tag="s1")
                nc.vector.scalar_tensor_tensor(
                    s1[:], dh[:, m, :], col(V_LNW2, m), pq[:, 0:NT],
                    op0=OP.mult, op1=OP.mult,
                )
                if lnb_zero:
                    nc.vector.scalar_tensor_tensor(
                        outf[:, m, :], pq[:, NT:], col(V_LNW2, m), s1[:],
                        op0=OP.mult, op1=OP.add,
                    )
                else:
                    s2 = ypool.tile([P, NT], f32, tag="s2")
                    nc.vector.scalar_tensor_tensor(
                        s2[:], pq[:, NT:], col(V_LNW2, m), s1[:],
                        op0=OP.mult, op1=OP.add,
                    )
                    nc.vector.tensor_scalar_add(outf[:, m, :], s2[:], col(V_LNB, m))
                if m % 2 == 1:  # stream results out as they complete
                    nc.sync.dma_start(
                        out=out_r[i, :, m - 1 : m + 1, :],
                        in_=outf[:, m - 1 : m + 1, :],
                    )
            state[i] = None

        for i in range(NB):
            matmul_phase(
                i,
                (lambda j=i: reduce_phase(j - 1)) if i > 0 else (lambda: None),
                (lambda j=i: bcast_phase(j - 1)) if i > 0 else (lambda: None),
            )
            if i > 0:
                epilogue_phase(i - 1)
        reduce_phase(NB - 1)
        bcast_phase(NB - 1)
        epilogue_phase(NB - 1)

    if not nc.is_finalized():
        nc.finalize()
    return nc


def _get_nc(lnb_zero):
    key = ("nc", lnb_zero)
    if key not in _CACHED:
        _CACHED[key] = _build_nc(lnb_zero)
    return _CACHED[key]


# test.py can flip these before calling kernel() to profile
TRACE = False
LAST_RESULT = {}


def kernel(t, h, e, W_rec, bias, tau, decay, ln_w, ln_b,
           ce_w1, ce_b1, ce_w2, ce_b2, pm_w, pm_b):
    from concourse.bass_utils import run_bass_kernel_spmd

    f = np.float32
    h = np.asarray(h, f)
    e = np.asarray(e, f)
    W_rec = np.asarray(W_rec, f)
    bias = np.asarray(bias, f)
    tau = np.asarray(tau, f)
    decay = np.asarray(decay, f)
    ln_w = np.asarray(ln_w, f)
    ln_b = np.asarray(ln_b, f)
    ce_w1 = np.asarray(ce_w1, f)
    ce_b1 = np.asarray(ce_b1, f)
    ce_w2 = np.asarray(ce_w2, f)
    ce_b2 = np.asarray(ce_b2, f)
    pm_w = np.asarray(pm_w, f)
    pm_b = np.asarray(pm_b, f)

    invtau = 1.0 / tau
    negdecay = -decay * invtau
    biasp = bias * invtau
    pmb_eff = pm_b + pm_w @ ce_b2  # fold ce_b2 through the param modulator
    lnb_zero = bool(np.all(ln_b == 0.0))

    w1T = np.ascontiguousarray(ce_w1.T).astype(BF16)
    w2T = np.ascontiguousarray(ce_w2.T).astype(BF16)
    pmT = np.ascontiguousarray(pm_w.T).astype(BF16)
    wrT = np.ascontiguousarray(W_rec.T * invtau[None, :]).astype(BF16)

    def chunked(v):  # [H] -> [128, KC] with column m = chunk m
        return np.ascontiguousarray(v.reshape(KC, P).T)

    consts = np.concatenate(
        [chunked(v) for v in (ce_b1, pmb_eff, negdecay, biasp, ln_w, ln_b)],
        axis=1,
    ).astype(f)

    def tile_major(x, rows):  # [BL, H] slice -> [NB, H, NT] transposed tiles
        return np.ascontiguousarray(
            x[rows].reshape(NB, NT, H).transpose(0, 2, 1)
        ).astype(BF16)

    in_maps = []
    for i in range(NCORES):
        rows = slice(i * BL, (i + 1) * BL)
        in_maps.append({
            "hT": tile_major(h, rows),
            "eT": tile_major(e, rows),
            "w1T": w1T, "w2T": w2T, "pmT": pmT, "wrT": wrT,
            "consts": consts,
        })

    nc = _get_nc(lnb_zero)
    res = run_bass_kernel_spmd(nc, in_maps, core_ids=list(range(NCORES)),
                               trace=TRACE)
    LAST_RESULT["exec_time_ns"] = res.exec_time_ns
    LAST_RESULT["mean_exec_time_ns"] = res.mean_exec_time_ns
    LAST_RESULT["instructions_and_trace"] = res.instructions_and_trace

    out = np.empty((B, H), f)
    for i in range(NCORES):
        # [NB, H, NT] tile-major transposed -> [BL, H]
        out[i * BL : (i + 1) * BL] = (
            res.results[i]["out"].transpose(0, 2, 1).reshape(BL, H)
        )
    return out


# revision 57
# speedup vs baseline: 1.0362x; 1.0015x over previous
"""AdaptiveLiquidNeuron forward on 8 TRN2 NeuronCores (data-parallel over batch).

Math (per batch row, H=1024):
  context = relu(h @ W1.T + b1) @ W2.T + b2
  pa      = context @ PM.T + pm_b
  mm      = (1 + pa) * (e @ Wrec.T)
  dh      = (-decay*h + mm + bias) / (tau * sigmoid(pa))
  out     = LayerNorm(dh) * ln_w + ln_b

Strategy: shard B=16384 over 8 cores (2048 rows each), replicate H x H weights.
On-chip everything is kept transposed ([H on partitions, B on free]) so the four
matmuls need no on-chip transposes (host pre-transposes weights + activations).
LayerNorm reduces over the partition axis via ones-matmuls (sum and sum-of-
squares side by side in one rhs); rstd = 2*Dsqrt(var+eps) with the 2 folded into
ln_w host-side; stats broadcast back across partitions with one K=1 matmul.
Host folds 1/tau into Wrec/decay/bias, ce_b2 into pm_b, and uses
1/sigmoid(x) = 1 + exp(-x).
"""

import numpy as np
import ml_dtypes

BF16 = ml_dtypes.bfloat16

B, H = 16384, 1024
NCORES = 8
BL = B // NCORES      # 2048 batch rows per core
P = 128               # partitions
KC = H // P           # 8 chunks of the hidden dim
NB = 8                # batch tiles per core
NT = BL // NB         # 256 batch columns per tile
EPS = 1e-5

# consts layout: [128, 6*KC] f32, column v*KC + m = chunk m of vector v
V_B1, V_PMB, V_NDEC, V_BIASP, V_LNW2, V_LNB = range(6)

_CACHED = {}


def _build_nc(lnb_zero):
    import concourse.bacc as bacc
    import concourse.tile as tile
    from concourse import mybir
    from contextlib import ExitStack

    f32 = mybir.dt.float32
    bf16 = mybir.dt.bfloat16
    AF = mybir.ActivationFunctionType
    OP = mybir.AluOpType

    nc = bacc.Bacc(target_bir_lowering=False)

    # h/e/out are tile-major on DRAM ([NB, H, NT]) so every DMA is contiguous
    hT_e = nc.declare_dram_parameter("hT", [NB, H, NT], bf16, isOutput=False)
    eT_e = nc.declare_dram_parameter("eT", [NB, H, NT], bf16, isOutput=False)
    w1_e = nc.declare_dram_parameter("w1T", [H, H], bf16, isOutput=False)
    w2_e = nc.declare_dram_parameter("w2T", [H, H], bf16, isOutput=False)
    pm_e = nc.declare_dram_parameter("pmT", [H, H], bf16, isOutput=False)
    wr_e = nc.declare_dram_parameter("wrT", [H, H], bf16, isOutput=False)
    cs_e = nc.declare_dram_parameter("consts", [P, 6 * KC], f32, isOutput=False)
    out_e = nc.declare_dram_parameter("out", [NB, H, NT], f32, isOutput=True)

    hT_r = hT_e[:].rearrange("i (k p) b -> i p k b", p=P)
    eT_r = eT_e[:].rearrange("i (k p) b -> i p k b", p=P)
    out_r = out_e[:].rearrange("i (m p) b -> i p m b", p=P)

    with tile.TileContext(nc) as tc, ExitStack() as ctx:
        wpool = ctx.enter_context(tc.tile_pool(name="weights", bufs=1))
        cpool = ctx.enter_context(tc.tile_pool(name="consts", bufs=1))
        iopool = ctx.enter_context(tc.tile_pool(name="io", bufs=3))
        actpool = ctx.enter_context(tc.tile_pool(name="acts", bufs=1))
        fpool = ctx.enter_context(tc.tile_pool(name="f32work", bufs=1))
        dhpool = ctx.enter_context(tc.tile_pool(name="dh", bufs=2))
        rpool = ctx.enter_context(tc.tile_pool(name="redu", bufs=2))
        ypool = ctx.enter_context(tc.tile_pool(name="y", bufs=4))
        rowpool = ctx.enter_context(tc.tile_pool(name="rows", bufs=2))
        outpool = ctx.enter_context(tc.tile_pool(name="outs", bufs=1))
        psA = ctx.enter_context(tc.tile_pool(name="psA", bufs=4, space="PSUM"))
        psS = ctx.enter_context(tc.tile_pool(name="psS", bufs=1, space="PSUM"))
        psB = ctx.enter_context(tc.tile_pool(name="psB", bufs=2, space="PSUM"))

        # ---- resident constants / weights ----
        # Prologue latency: mm1 needs consts+w1+hT0 first — split those
        # halves across the sync and gpsimd DMA queues so they stream in
        # parallel; everything else queues up behind in need-order.
        consts = cpool.tile([P, 6 * KC], f32, tag="consts")
        nc.gpsimd.dma_start(out=consts[:], in_=cs_e[:])

        def col(v, m):
            return consts[:, v * KC + m : v * KC + m + 1]

        w_sb = {}
        for nm, ext in (("w1", w1_e), ("w2", w2_e), ("pm", pm_e), ("wr", wr_e)):
            w_sb[nm] = (wpool.tile([P, KC, H], bf16, tag=nm, name=f"w_{nm}"), ext)

        def load_w(nm, eng, lo=0, hi=KC):
            t, ext = w_sb[nm]
            src = ext[:].rearrange("(k p) m -> p k m", p=P)
            eng.dma_start(out=t[:, lo:hi, :], in_=src[:, lo:hi, :])
            return t

        def load_io(i, h_eng, e_eng, split=False):
            ht = iopool.tile([P, KC, NT], bf16, tag="hT")
            et = iopool.tile([P, KC, NT], bf16, tag="eT")
            if split:
                h_eng.dma_start(out=ht[:, 0 : KC // 2, :],
                                in_=hT_r[i, :, 0 : KC // 2, :])
                e_eng.dma_start(out=ht[:, KC // 2 :, :],
                                in_=hT_r[i, :, KC // 2 :, :])
            else:
                h_eng.dma_start(out=ht[:], in_=hT_r[i])
            e_eng.dma_start(out=et[:], in_=eT_r[i])
            return ht, et

        # Prologue: ~1MB pieces (descriptor-efficient) spread over the two
        # HWDGE rings (SP, ACT) + SWDGE (gpsimd) in the order compute needs
        # them: w1+h0 (mm1), w2 (mm2), pm (mm3), wr+e0 (mm4), then tile 1.
        ht0 = iopool.tile([P, KC, NT], bf16, tag="hT")
        et0 = iopool.tile([P, KC, NT], bf16, tag="eT")
        w1_sb = w_sb["w1"][0]
        for k in range(0, KC, 2):
            load_w("w1", nc.sync if k % 4 == 0 else nc.scalar, k, k + 2)
            (nc.scalar if k % 4 == 0 else nc.sync).dma_start(
                out=ht0[:, k : k + 2, :], in_=hT_r[0, :, k : k + 2, :]
            )
        w2_sb = load_w("w2", nc.sync, 0, 4)
        load_w("w2", nc.scalar, 4, 8)
        pm_sb = load_w("pm", nc.sync, 0, 4)
        load_w("pm", nc.scalar, 4, 8)
        wr_sb = load_w("wr", nc.sync, 0, 4)
        load_w("wr", nc.scalar, 4, 8)
        nc.gpsimd.dma_start(out=et0[:], in_=eT_r[0])
        io_tiles = [(ht0, et0), None]
        io_tiles[1] = load_io(1, nc.gpsimd, nc.gpsimd)

        ones_col = cpool.tile([P, 1], bf16, tag="ones_col")
        nc.vector.memset(ones_col[:], 1.0)
        ones_row = cpool.tile([1, P], f32, tag="ones_row")
        nc.vector.memset(ones_row[:], 1.0)

        # dummy matmuls during the prologue DMA wait: PE-HAM sees ~4us of
        # sustained activity and unthrottles to 2.4GHz before real work
        warm_w = cpool.tile([P, P], bf16, tag="warm_w")
        warm_x = cpool.tile([P, NT], bf16, tag="warm_x")
        nc.vector.memset(warm_w[:], 0.0)
        nc.vector.memset(warm_x[:], 0.0)
        warm_ps = psS.tile([1, 2 * NT], f32, tag="sums", name="warm_ps")
        for _ in range(40):
            nc.tensor.matmul(warm_ps[:, 0:NT], warm_w[:, 0:1], warm_x[:],
                             start=True, stop=True)


        state = [None] * NB

        def mm_layer(w, rhs_t, evac):
            """psum[m] = w[:,:,m].T @ rhs (contract KC chunks); evac(m, psum)."""
            for m in range(KC):
                acc = psA.tile([P, NT], f32, tag="acc")
                for k in range(KC):
                    nc.tensor.matmul(
                        acc[:],
                        w[:, k, m * P : (m + 1) * P],
                        rhs_t[:, k, :],
                        start=(k == 0),
                        stop=(k == KC - 1),
                    )
                evac(m, acc)

        def matmul_phase(i, pe_hook1, pe_hook2):
            ht, et = io_tiles[i % 2]
            if i + 2 < NB:
                io_tiles[i % 2] = load_io(i + 2, nc.sync, nc.sync)

            c1 = actpool.tile([P, KC, NT], bf16, tag="c1")
            cx = actpool.tile([P, KC, NT], bf16, tag="ctx")
            pa = fpool.tile([P, KC, NT], f32, tag="pa")
            ex = fpool.tile([P, KC, NT], f32, tag="exp")
            t2 = fpool.tile([P, KC, NT], f32, tag="t2")
            u = fpool.tile([P, KC, NT], f32, tag="u")
            num = fpool.tile([P, KC, NT], f32, tag="num")
            dh = dhpool.tile([P, KC, NT], f32, tag="dh")
            # dh (bf16) and dh^2 side by side so one ones-matmul per chunk
            # yields both sum and sum-of-squares
            red = rpool.tile([P, KC, 2 * NT], bf16, tag="red")

            # u only needs hT + consts: emit first so it clears the in-order
            # gpsimd queue before tile i-1's row math lands there
            for m in range(KC):
                nc.gpsimd.tensor_scalar(
                    u[:, m, :],
                    ht[:, m, :],
                    col(V_NDEC, m),
                    col(V_BIASP, m),
                    op0=OP.mult,
                    op1=OP.add,
                )

            # context encoder layer 1: c1 = relu(W1 @ hT + b1)
            def relu_evac(m, acc):
                nc.scalar.activation(
                    c1[:, m, :], acc[:], AF.Relu, bias=col(V_B1, m), scale=1.0
                )

            if i == 0:
                # k-outer in m-halves: consumes w1/hT chunks as the DMAs
                # land instead of waiting for the full tensors
                for half in range(2):
                    ms_ = range(half * 4, half * 4 + 4)
                    accs = [
                        psA.tile([P, NT], f32, tag="acc", name=f"acc0_{m}")
                        for m in ms_
                    ]
                    for k in range(KC):
                        for j, m in enumerate(ms_):
                            nc.tensor.matmul(
                                accs[j][:],
                                w1_sb[:, k, m * P : (m + 1) * P],
                                ht[:, k, :],
                                start=(k == 0),
                                stop=(k == KC - 1),
                            )
                    for j, m in enumerate(ms_):
                        relu_evac(m, accs[j])
            else:
                mm_layer(w1_sb, ht, relu_evac)
            pe_hook1()  # reductions of tile i-1 slot in here on PE
            # context encoder layer 2 (b2 folded into pm_b): ctx = W2 @ c1
            mm_layer(
                w2_sb,
                c1,
                lambda m, acc: nc.scalar.activation(
                    cx[:, m, :], acc[:], AF.Copy, bias=0.0, scale=1.0
                ),
            )
            pe_hook2()  # stat broadcast of tile i-1
            # param modulator: pa = PM @ ctx + pm_b'
            mm_layer(
                pm_sb,
                cx,
                lambda m, acc: nc.vector.tensor_scalar_add(
                    pa[:, m, :], acc[:], col(V_PMB, m)
                ),
            )
            # 1/sigmoid(pa) = 1 + exp(-pa)
            nc.scalar.activation(ex[:], pa[:], AF.Exp, bias=0.0, scale=-1.0)

            # recurrent: t2 = (1 + pa) * (Wrec' @ eT)
            mm_layer(
                wr_sb,
                et,
                lambda m, acc: nc.vector.scalar_tensor_tensor(
                    t2[:, m, :], pa[:, m, :], 1.0, acc[:], op0=OP.add, op1=OP.mult
                ),
            )

            # halves keep the tail latency down: reduce matmuls for half 0
            # can start while half 1 is still in the vector pipe
            nsplit = 4 if i == NB - 1 else 2  # short tail for the last tile
            step = KC // nsplit
            for s in [slice(j * step, (j + 1) * step) for j in range(nsplit)]:
                nc.vector.tensor_add(num[:, s, :], t2[:, s, :], u[:, s, :])
                # dh = num * (1 + exp(-pa))
                nc.vector.scalar_tensor_tensor(
                    dh[:, s, :], ex[:, s, :], 1.0, num[:, s, :],
                    op0=OP.add, op1=OP.mult,
                )
                nc.scalar.square(red[:, s, NT:], dh[:, s, :])
                nc.scalar.copy(red[:, s, 0:NT], dh[:, s, :])
            state[i] = (dh, red)

        def reduce_phase(i):
            # partition-axis sum+sumsq via ones-matmuls over all H=1024
            dh, red = state[i]
            s_ps = psS.tile([1, 2 * NT], f32, tag="sums")
            for m in range(KC):
                nc.tensor.matmul(
                    s_ps[:], ones_col[:], red[:, m, :],
                    start=(m == 0), stop=(m == KC - 1),
                )
            i32 = mybir.dt.int32
            mu_n = rowpool.tile([1, NT], f32, tag="mu_n")
            ms = rowpool.tile([1, NT], f32, tag="ms")
            musq = rowpool.tile([1, NT], f32, tag="musq")
            ve = rowpool.tile([1, NT], f32, tag="ve")
            yb = rowpool.tile([1, NT], f32, tag="yb")
            t1 = rowpool.tile([1, NT], f32, tag="t1")
            t2r = rowpool.tile([1, NT], f32, tag="t2r")
            dq = rowpool.tile([1, 2 * NT], f32, tag="dq")
            # rstd = rsqrt(var+eps) via Quake initial guess + one Newton
            # step (rel err ~2e-3, far below bf16 matmul noise) — avoids
            # ln/sqrt ACT funcs so the whole kernel stays in one
            # activation-table set (no table reloads)
            nc.vector.tensor_scalar_mul(mu_n[:], s_ps[:, 0:NT], -1.0 / H)
            nc.vector.tensor_scalar(ms[:], s_ps[:, NT:], 1.0 / H, EPS,
                                    op0=OP.mult, op1=OP.add)
            nc.vector.tensor_mul(musq[:], mu_n[:], mu_n[:])
            nc.vector.tensor_sub(ve[:], ms[:], musq[:])  # var + eps
            nc.vector.tensor_scalar(
                t1[:].bitcast(i32), ve[:].bitcast(i32), 1, None,
                op0=OP.arith_shift_right,
            )
            nc.vector.tensor_scalar(
                yb[:].bitcast(i32), t1[:].bitcast(i32), -1, 0x5F3759DF,
                op0=OP.mult, op1=OP.add,
            )
            # y1 = y0*(1.5 - 0.5*ve*y0^2)
            nc.vector.tensor_mul(t1[:], yb[:], yb[:])
            nc.vector.tensor_mul(t2r[:], t1[:], ve[:])
            nc.vector.tensor_scalar(t2r[:], t2r[:], -0.5, 1.5,
                                    op0=OP.mult, op1=OP.add)
            nc.vector.tensor_mul(dq[:, 0:NT], yb[:], t2r[:])
            nc.vector.tensor_mul(dq[:, NT:], mu_n[:], dq[:, 0:NT])
            state[i] = (dh, dq)

        def bcast_phase(i):
            dh, dq = state[i]
            pq = psB.tile([P, 2 * NT], f32, tag="PQ")
            nc.tensor.matmul(pq[:], ones_row[:], dq[:], start=True, stop=True)
            state[i] = (dh, pq)

        def epilogue_phase(i):
            dh, pq = state[i]
            outf = outpool.tile([P, KC, NT], f32, tag="outf")
            for m in range(KC):
                # out = lnw2*(dh*D + qn) (+ lnb) = ln_w*rstd*(dh-mu) + ln_b
                s1 = ypool.tile([P, NT], f32, tag="s1")
                nc.vector.scalar_tensor_tensor(
                    s1[:], dh[:, m, :], col(V_LNW2, m), pq[:, 0:NT],
                    op0=OP.mult, op1=OP.mult,
                )
                if lnb_zero:
                    nc.vector.scalar_tensor_tensor(
                        outf[:, m, :], pq[:, NT:], col(V_LNW2, m), s1[:],
                        op0=OP.mult, op1=OP.add,
                    )
                else:
                    s2 = ypool.tile([P, NT], f32, tag="s2")
                    nc.vector.scalar_tensor_tensor(
                        s2[:], pq[:, NT:], col(V_LNW2, m), s1[:],
                        op0=OP.mult, op1=OP.add,
                    )
                    nc.vector.tensor_scalar_add(outf[:, m, :], s2[:], col(V_LNB, m))
                if m % 2 == 1:  # stream results out as they complete
                    nc.sync.dma_start(
                        out=out_r[i, :, m - 1 : m + 1, :],
                        in_=outf[:, m - 1 : m + 1, :],
                    )
            state[i] = None

        for i in range(NB):
            matmul_phase(
                i,
                (lambda j=i: reduce_phase(j - 1)) if i > 0 else (lambda: None),
                (lambda j=i: bcast_phase(j - 1)) if i > 0 else (lambda: None),
            )
            if i > 0:
                epilogue_phase(i - 1)
        reduce_phase(NB - 1)
        bcast_phase(NB - 1)
        epilogue_phase(NB - 1)

    if not nc.is_finalized():
        nc.finalize()
    return nc


def _get_nc(lnb_zero):
    key = ("nc", lnb_zero)
    if key not in _CACHED:
        _CACHED[key] = _build_nc(lnb_zero)
    return _CACHED[key]


# test.py can flip these before calling kernel() to profile
TRACE = False
LAST_RESULT = {}


def kernel(t, h, e, W_rec, bias, tau, decay, ln_w, ln_b,
           ce_w1, ce_b1, ce_w2, ce_b2, pm_w, pm_b):
    from concourse.bass_utils import run_bass_kernel_spmd

    f = np.float32
    h = np.asarray(h, f)
    e = np.asarray(e, f)
    W_rec = np.asarray(W_rec, f)
    bias = np.asarray(bias, f)
    tau = np.asarray(tau, f)
    decay = np.asarray(decay, f)
    ln_w = np.asarray(ln_w, f)
    ln_b = np.asarray(ln_b, f)
    ce_w1 = np.asarray(ce_w1, f)
    ce_b1 = np.asarray(ce_b1, f)
    ce_w2 = np.asarray(ce_w2, f)
    ce_b2 = np.asarray(ce_b2, f)
    pm_w = np.asarray(pm_w, f)
    pm_b = np.asarray(pm_b, f)

    invtau = 1.0 / tau
    negdecay = -decay * invtau
    biasp = bias * invtau
    pmb_eff = pm_b + pm_w @ ce_b2  # fold ce_b2 through the param modulator
    lnb_zero = bool(np.all(ln_b == 0.0))

    w1T = np.ascontiguousarray(ce_w1.T).astype(BF16)
    w2T = np.ascontiguousarray(ce_w2.T).astype(BF16)
    pmT = np.ascontiguousarray(pm_w.T).astype(BF16)
    wrT = np.ascontiguousarray(W_rec.T * invtau[None, :]).astype(BF16)

    def chunked(v):  # [H] -> [128, KC] with column m = chunk m
        return np.ascontiguousarray(v.reshape(KC, P).T)

    consts = np.concatenate(
        [chunked(v) for v in (ce_b1, pmb_eff, negdecay, biasp, ln_w, ln_b)],
        axis=1,
    ).astype(f)

    def tile_major(x, rows):  # [BL, H] slice -> [NB, H, NT] transposed tiles
        return np.ascontiguousarray(
            x[rows].reshape(NB, NT, H).transpose(0, 2, 1)
        ).astype(BF16)

    in_maps = []
    for i in range(NCORES):
        rows = slice(i * BL, (i + 1) * BL)
        in_maps.append({
            "hT": tile_major(h, rows),
            "eT": tile_major(e, rows),
            "w1T": w1T, "w2T": w2T, "pmT": pmT, "wrT": wrT,
            "consts": consts,
        })

    nc = _get_nc(lnb_zero)
    res = run_bass_kernel_spmd(nc, in_maps, core_ids=list(range(NCORES)),
                               trace=TRACE)
    LAST_RESULT["exec_time_ns"] = res.exec_time_ns
    LAST_RESULT["mean_exec_time_ns"] = res.mean_exec_time_ns
    LAST_RESULT["instructions_and_trace"] = res.instructions_and_trace

    out = np.empty((B, H), f)
    for i in range(NCORES):
        # [NB, H, NT] tile-major transposed -> [BL, H]
        out[i * BL : (i + 1) * BL] = (
            res.results[i]["out"].transpose(0, 2, 1).reshape(BL, H)
        )
    return out


# revision 62
# speedup vs baseline: 1.0399x; 1.0035x over previous
"""AdaptiveLiquidNeuron forward on 8 TRN2 NeuronCores (data-parallel over batch).

Math (per batch row, H=1024):
  context = relu(h @ W1.T + b1) @ W2.T + b2
  pa      = context @ PM.T + pm_b
  mm      = (1 + pa) * (e @ Wrec.T)
  dh      = (-decay*h + mm + bias) / (tau * sigmoid(pa))
  out     = LayerNorm(dh) * ln_w + ln_b

Strategy: shard B=16384 over 8 cores (2048 rows each), replicate H x H weights.
On-chip everything is kept transposed ([H on partitions, B on free]) so the four
matmuls need no on-chip transposes (host pre-transposes weights + activations).
LayerNorm reduces over the partition axis via ones-matmuls (sum and sum-of-
squares side by side in one rhs); rstd = 2*Dsqrt(var+eps) with the 2 folded into
ln_w host-side; stats broadcast back across partitions with one K=1 matmul.
Host folds 1/tau into Wrec/decay/bias, ce_b2 into pm_b, and uses
1/sigmoid(x) = 1 + exp(-x).
"""

import numpy as np
import ml_dtypes

BF16 = ml_dtypes.bfloat16

B, H = 16384, 1024
NCORES = 8
BL = B // NCORES      # 2048 batch rows per core
P = 128               # partitions
KC = H // P           # 8 chunks of the hidden dim
NB = 8                # batch tiles per core
NT = BL // NB         # 256 batch columns per tile
EPS = 1e-5

# consts layout: [128, 6*KC] f32, column v*KC + m = chunk m of vector v
V_B1, V_PMB, V_NDEC, V_BIASP, V_LNW2, V_LNB = range(6)

_CACHED = {}


def _build_nc(lnb_zero):
    import concourse.bass as bass
    import concourse.bacc as bacc
    import concourse.tile as tile
    from concourse import mybir
    from contextlib import ExitStack

    f32 = mybir.dt.float32
    bf16 = mybir.dt.bfloat16
    AF = mybir.ActivationFunctionType
    OP = mybir.AluOpType

    nc = bacc.Bacc(target_bir_lowering=False)

    # h/e/out are tile-major on DRAM ([NB, H, NT]) so every DMA is contiguous
    hT_e = nc.declare_dram_parameter("hT", [NB, H, NT], bf16, isOutput=False)
    eT_e = nc.declare_dram_parameter("eT", [NB, H, NT], bf16, isOutput=False)
    w1_e = nc.declare_dram_parameter("w1T", [H, H], bf16, isOutput=False)
    w2_e = nc.declare_dram_parameter("w2T", [H, H], bf16, isOutput=False)
    pm_e = nc.declare_dram_parameter("pmT", [H, H], bf16, isOutput=False)
    wr_e = nc.declare_dram_parameter("wrT", [H, H], bf16, isOutput=False)
    cs_e = nc.declare_dram_parameter("consts", [P, 6 * KC], f32, isOutput=False)
    out_e = nc.declare_dram_parameter("out", [NB, H, NT], f32, isOutput=True)

    hT_r = hT_e[:].rearrange("i (k p) b -> i p k b", p=P)
    eT_r = eT_e[:].rearrange("i (k p) b -> i p k b", p=P)
    out_r = out_e[:].rearrange("i (m p) b -> i p m b", p=P)

    with tile.TileContext(nc) as tc, ExitStack() as ctx:
        wpool = ctx.enter_context(tc.tile_pool(name="weights", bufs=1))
        cpool = ctx.enter_context(tc.tile_pool(name="consts", bufs=1))
        iopool = ctx.enter_context(tc.tile_pool(name="io", bufs=3))
        actpool = ctx.enter_context(tc.tile_pool(name="acts", bufs=1))
        fpool = ctx.enter_context(tc.tile_pool(name="f32work", bufs=1))
        dhpool = ctx.enter_context(tc.tile_pool(name="dh", bufs=2))
        rpool = ctx.enter_context(tc.tile_pool(name="redu", bufs=2))
        ypool = ctx.enter_context(tc.tile_pool(name="y", bufs=4))
        rowpool = ctx.enter_context(tc.tile_pool(name="rows", bufs=2))
        outpool = ctx.enter_context(tc.tile_pool(name="outs", bufs=1))
        bcpool = ctx.enter_context(tc.tile_pool(name="bc", bufs=2))
        drampool = ctx.enter_context(tc.tile_pool(name="dram", bufs=2,
                                                  space="DRAM"))
        psA = ctx.enter_context(tc.tile_pool(name="psA", bufs=5, space="PSUM"))
        psS = ctx.enter_context(tc.tile_pool(name="psS", bufs=1, space="PSUM"))

        # ---- resident constants / weights ----
        # Prologue latency: mm1 needs consts+w1+hT0 first — split those
        # halves across the sync and gpsimd DMA queues so they stream in
        # parallel; everything else queues up behind in need-order.
        consts = cpool.tile([P, 6 * KC], f32, tag="consts")
        nc.gpsimd.dma_start(out=consts[:], in_=cs_e[:])

        def col(v, m):
            return consts[:, v * KC + m : v * KC + m + 1]

        w_sb = {}
        for nm, ext in (("w1", w1_e), ("w2", w2_e), ("pm", pm_e), ("wr", wr_e)):
            w_sb[nm] = (wpool.tile([P, KC, H], bf16, tag=nm, name=f"w_{nm}"), ext)

        def load_w(nm, eng, lo=0, hi=KC):
            t, ext = w_sb[nm]
            src = ext[:].rearrange("(k p) m -> p k m", p=P)
            eng.dma_start(out=t[:, lo:hi, :], in_=src[:, lo:hi, :])
            return t

        def load_io(i, h_eng, e_eng, split=False):
            ht = iopool.tile([P, KC, NT], bf16, tag="hT")
            et = iopool.tile([P, KC, NT], bf16, tag="eT")
            if split:
                h_eng.dma_start(out=ht[:, 0 : KC // 2, :],
                                in_=hT_r[i, :, 0 : KC // 2, :])
                e_eng.dma_start(out=ht[:, KC // 2 :, :],
                                in_=hT_r[i, :, KC // 2 :, :])
            else:
                h_eng.dma_start(out=ht[:], in_=hT_r[i])
            e_eng.dma_start(out=et[:], in_=eT_r[i])
            return ht, et

        # Prologue: ~1MB pieces (descriptor-efficient) spread over the two
        # HWDGE rings (SP, ACT) + SWDGE (gpsimd) in the order compute needs
        # them: w1+h0 (mm1), w2 (mm2), pm (mm3), wr+e0 (mm4), then tile 1.
        ht0 = iopool.tile([P, KC, NT], bf16, tag="hT")
        et0 = iopool.tile([P, KC, NT], bf16, tag="eT")
        w1_sb = w_sb["w1"][0]
        for k in range(0, KC, 2):
            load_w("w1", nc.sync if k % 4 == 0 else nc.scalar, k, k + 2)
            (nc.scalar if k % 4 == 0 else nc.sync).dma_start(
                out=ht0[:, k : k + 2, :], in_=hT_r[0, :, k : k + 2, :]
            )
        w2_sb = load_w("w2", nc.sync, 0, 4)
        load_w("w2", nc.scalar, 4, 8)
        pm_sb = load_w("pm", nc.sync, 0, 4)
        load_w("pm", nc.scalar, 4, 8)
        wr_sb = load_w("wr", nc.sync, 0, 4)
        load_w("wr", nc.scalar, 4, 8)
        nc.gpsimd.dma_start(out=et0[:], in_=eT_r[0])
        io_tiles = [(ht0, et0), None]
        io_tiles[1] = load_io(1, nc.gpsimd, nc.gpsimd)

        ones_col = cpool.tile([P, 1], bf16, tag="ones_col")
        nc.vector.memset(ones_col[:], 1.0)
        ones_row = cpool.tile([1, P], f32, tag="ones_row")
        nc.vector.memset(ones_row[:], 1.0)

        # dummy matmuls during the prologue DMA wait: PE-HAM sees ~4us of
        # sustained activity and unthrottles to 2.4GHz before real work
        warm_w = cpool.tile([P, P], bf16, tag="warm_w")
        warm_x = cpool.tile([P, NT], bf16, tag="warm_x")
        nc.vector.memset(warm_w[:], 0.0)
        nc.vector.memset(warm_x[:], 0.0)
        warm_ps = psS.tile([1, 2 * NT], f32, tag="sums", name="warm_ps")
        for _ in range(40):
            nc.tensor.matmul(warm_ps[:, 0:NT], warm_w[:, 0:1], warm_x[:],
                             start=True, stop=True)


        state = [None] * NB

        def mm_layer(w, rhs_t, evac):
            """psum[m] = w[:,:,m].T @ rhs (contract KC chunks); evac(m, psum)."""
            for m in range(KC):
                acc = psA.tile([P, NT], f32, tag="acc")
                for k in range(KC):
                    nc.tensor.matmul(
                        acc[:],
                        w[:, k, m * P : (m + 1) * P],
                        rhs_t[:, k, :],
                        start=(k == 0),
                        stop=(k == KC - 1),
                    )
                evac(m, acc)

        def matmul_phase(i, pe_hook1, pe_hook2):
            ht, et = io_tiles[i % 2]
            if i + 2 < NB:
                io_tiles[i % 2] = load_io(i + 2, nc.sync, nc.sync)

            c1 = actpool.tile([P, KC, NT], bf16, tag="c1")
            cx = actpool.tile([P, KC, NT], bf16, tag="ctx")
            pa = fpool.tile([P, KC, NT], f32, tag="pa")
            ex = fpool.tile([P, KC, NT], f32, tag="exp")
            t2 = fpool.tile([P, KC, NT], f32, tag="t2")
            u = fpool.tile([P, KC, NT], f32, tag="u")
            num = fpool.tile([P, KC, NT], f32, tag="num")
            dh = dhpool.tile([P, KC, NT], f32, tag="dh")
            # dh (bf16) and dh^2 side by side so one ones-matmul per chunk
            # yields both sum and sum-of-squares
            red = rpool.tile([P, KC, 2 * NT], bf16, tag="red")

            # u only needs hT + consts: emit first so it clears the in-order
            # gpsimd queue before tile i-1's row math lands there
            for m in range(KC):
                nc.gpsimd.tensor_scalar(
                    u[:, m, :],
                    ht[:, m, :],
                    col(V_NDEC, m),
                    col(V_BIASP, m),
                    op0=OP.mult,
                    op1=OP.add,
                )

            # context encoder layer 1: c1 = relu(W1 @ hT + b1)
            def relu_evac(m, acc):
                nc.scalar.activation(
                    c1[:, m, :], acc[:], AF.Relu, bias=col(V_B1, m), scale=1.0
                )

            if i == 0:
                # k-outer in m-halves: consumes w1/hT chunks as the DMAs
                # land instead of waiting for the full tensors
                for half in range(2):
                    ms_ = range(half * 4, half * 4 + 4)
                    accs = [
                        psA.tile([P, NT], f32, tag="acc", name=f"acc0_{m}")
                        for m in ms_
                    ]
                    for k in range(KC):
                        for j, m in enumerate(ms_):
                            nc.tensor.matmul(
                                accs[j][:],
                                w1_sb[:, k, m * P : (m + 1) * P],
                                ht[:, k, :],
                                start=(k == 0),
                                stop=(k == KC - 1),
                            )
                    for j, m in enumerate(ms_):
                        relu_evac(m, accs[j])
            else:
                mm_layer(w1_sb, ht, relu_evac)
            pe_hook1()  # reductions of tile i-1 slot in here on PE
            # context encoder layer 2 (b2 folded into pm_b): ctx = W2 @ c1
            mm_layer(
                w2_sb,
                c1,
                lambda m, acc: nc.scalar.activation(
                    cx[:, m, :], acc[:], AF.Copy, bias=0.0, scale=1.0
                ),
            )
            pe_hook2()  # stat broadcast of tile i-1
            # param modulator: pa = PM @ ctx + pm_b'
            mm_layer(
                pm_sb,
                cx,
                lambda m, acc: nc.vector.tensor_scalar_add(
                    pa[:, m, :], acc[:], col(V_PMB, m)
                ),
            )
            # 1/sigmoid(pa) = 1 + exp(-pa)
            nc.scalar.activation(ex[:], pa[:], AF.Exp, bias=0.0, scale=-1.0)

            # recurrent: t2 = (1 + pa) * (Wrec' @ eT)
            mm_layer(
                wr_sb,
                et,
                lambda m, acc: nc.vector.scalar_tensor_tensor(
                    t2[:, m, :], pa[:, m, :], 1.0, acc[:], op0=OP.add, op1=OP.mult
                ),
            )

            # halves keep the tail latency down: reduce matmuls for half 0
            # can start while half 1 is still in the vector pipe
            nsplit = 4 if i == NB - 1 else 2  # short tail for the last tile
            step = KC // nsplit
            for s in [slice(j * step, (j + 1) * step) for j in range(nsplit)]:
                nc.vector.tensor_add(num[:, s, :], t2[:, s, :], u[:, s, :])
                # dh = num * (1 + exp(-pa))
                nc.vector.scalar_tensor_tensor(
                    dh[:, s, :], ex[:, s, :], 1.0, num[:, s, :],
                    op0=OP.add, op1=OP.mult,
                )
                nc.scalar.square(red[:, s, NT:], dh[:, s, :])
                nc.scalar.copy(red[:, s, 0:NT], dh[:, s, :])
            state[i] = (dh, red)

        def reduce_phase(i):
            # partition-axis sum+sumsq via ones-matmuls over all H=1024
            dh, red = state[i]
            s_ps = psS.tile([1, 2 * NT], f32, tag="sums")
            for m in range(KC):
                nc.tensor.matmul(
                    s_ps[:], ones_col[:], red[:, m, :],
                    start=(m == 0), stop=(m == KC - 1),
                )
            i32 = mybir.dt.int32
            mu_n = rowpool.tile([1, NT], f32, tag="mu_n")
            ms = rowpool.tile([1, NT], f32, tag="ms")
            musq = rowpool.tile([1, NT], f32, tag="musq")
            ve = rowpool.tile([1, NT], f32, tag="ve")
            yb = rowpool.tile([1, NT], f32, tag="yb")
            t1 = rowpool.tile([1, NT], f32, tag="t1")
            t2r = rowpool.tile([1, NT], f32, tag="t2r")
            dq = rowpool.tile([1, 2 * NT], f32, tag="dq")
            # rstd = rsqrt(var+eps) via Quake initial guess + one Newton
            # step (rel err ~2e-3, far below bf16 matmul noise) — avoids
            # ln/sqrt ACT funcs so the whole kernel stays in one
            # activation-table set (no table reloads)
            nc.vector.tensor_scalar_mul(mu_n[:], s_ps[:, 0:NT], -1.0 / H)
            nc.vector.tensor_scalar(ms[:], s_ps[:, NT:], 1.0 / H, EPS,
                                    op0=OP.mult, op1=OP.add)
            nc.vector.tensor_mul(musq[:], mu_n[:], mu_n[:])
            nc.vector.tensor_sub(ve[:], ms[:], musq[:])  # var + eps
            nc.vector.tensor_scalar(
                t1[:].bitcast(i32), ve[:].bitcast(i32), 1, None,
                op0=OP.arith_shift_right,
            )
            nc.vector.tensor_scalar(
                yb[:].bitcast(i32), t1[:].bitcast(i32), -1, 0x5F3759DF,
                op0=OP.mult, op1=OP.add,
            )
            # y1 = y0*(1.5 - 0.5*ve*y0^2)
            nc.vector.tensor_mul(t1[:], yb[:], yb[:])
            nc.vector.tensor_mul(t2r[:], t1[:], ve[:])
            nc.vector.tensor_scalar(t2r[:], t2r[:], -0.5, 1.5,
                                    op0=OP.mult, op1=OP.add)
            nc.vector.tensor_mul(dq[:, 0:NT], yb[:], t2r[:])
            nc.vector.tensor_mul(dq[:, NT:], mu_n[:], dq[:, 0:NT])
            state[i] = (dh, dq)

        def bcast_phase(i):
            # broadcast the [1, 2*NT] stats row across partitions via a DRAM
            # bounce + partition-step-0 read — costs no PE time
            dh, dq = state[i]
            dqd = drampool.tile([2 * NT], f32, tag="dqd")
            nc.sync.dma_start(out=dqd[:], in_=dq[:])
            pq = bcpool.tile([P, 2 * NT], f32, tag="PQ")
            src = bass.AP(tensor=dqd.tensor, offset=dqd.offset,
                          ap=[[0, P]] + [list(a) for a in dqd.ap])
            nc.sync.dma_start(out=pq[:], in_=src)
            state[i] = (dh, pq)

        def epilogue_phase(i):
            dh, pq = state[i]
            outf = outpool.tile([P, KC, NT], f32, tag="outf")
            for m in range(KC):
                # out = lnw2*(dh*D + qn) (+ lnb) = ln_w*rstd*(dh-mu) + ln_b
                s1 = ypool.tile([P, NT], f32, tag="s1")
                nc.vector.scalar_tensor_tensor(
                    s1[:], dh[:, m, :], col(V_LNW2, m), pq[:, 0:NT],
                    op0=OP.mult, op1=OP.mult,
                )
                if lnb_zero:
                    nc.vector.scalar_tensor_tensor(
                        outf[:, m, :], pq[:, NT:], col(V_LNW2, m), s1[:],
                        op0=OP.mult, op1=OP.add,
                    )
                else:
                    s2 = ypool.tile([P, NT], f32, tag="s2")
                    nc.vector.scalar_tensor_tensor(
                        s2[:], pq[:, NT:], col(V_LNW2, m), s1[:],
                        op0=OP.mult, op1=OP.add,
                    )
                    nc.vector.tensor_scalar_add(outf[:, m, :], s2[:], col(V_LNB, m))
                if m % 2 == 1:  # stream results out as they complete
                    nc.sync.dma_start(
                        out=out_r[i, :, m - 1 : m + 1, :],
                        in_=outf[:, m - 1 : m + 1, :],
                    )
            state[i] = None

        for i in range(NB):
            matmul_phase(
                i,
                (lambda j=i: reduce_phase(j - 1)) if i > 0 else (lambda: None),
                (lambda j=i: bcast_phase(j - 1)) if i > 0 else (lambda: None),
            )
            if i > 0:
                epilogue_phase(i - 1)
        reduce_phase(NB - 1)
        bcast_phase(NB - 1)
        epilogue_phase(NB - 1)

    if not nc.is_finalized():
        nc.finalize()
    return nc


def _get_nc(lnb_zero):
    key = ("nc", lnb_zero)
    if key not in _CACHED:
        _CACHED[key] = _build_nc(lnb_zero)
    return _CACHED[key]


# test.py can flip these before calling kernel() to profile
TRACE = False
LAST_RESULT = {}


def kernel(t, h, e, W_rec, bias, tau, decay, ln_w, ln_b,
           ce_w1, ce_b1, ce_w2, ce_b2, pm_w, pm_b):
    from concourse.bass_utils import run_bass_kernel_spmd

    f = np.float32
    h = np.asarray(h, f)
    e = np.asarray(e, f)
    W_rec = np.asarray(W_rec, f)
    bias = np.asarray(bias, f)
    tau = np.asarray(tau, f)
    decay = np.asarray(decay, f)
    ln_w = np.asarray(ln_w, f)
    ln_b = np.asarray(ln_b, f)
    ce_w1 = np.asarray(ce_w1, f)
    ce_b1 = np.asarray(ce_b1, f)
    ce_w2 = np.asarray(ce_w2, f)
    ce_b2 = np.asarray(ce_b2, f)
    pm_w = np.asarray(pm_w, f)
    pm_b = np.asarray(pm_b, f)

    invtau = 1.0 / tau
    negdecay = -decay * invtau
    biasp = bias * invtau
    pmb_eff = pm_b + pm_w @ ce_b2  # fold ce_b2 through the param modulator
    lnb_zero = bool(np.all(ln_b == 0.0))

    w1T = np.ascontiguousarray(ce_w1.T).astype(BF16)
    w2T = np.ascontiguousarray(ce_w2.T).astype(BF16)
    pmT = np.ascontiguousarray(pm_w.T).astype(BF16)
    wrT = np.ascontiguousarray(W_rec.T * invtau[None, :]).astype(BF16)

    def chunked(v):  # [H] -> [128, KC] with column m = chunk m
        return np.ascontiguousarray(v.reshape(KC, P).T)

    consts = np.concatenate(
        [chunked(v) for v in (ce_b1, pmb_eff, negdecay, biasp, ln_w, ln_b)],
        axis=1,
    ).astype(f)

    def tile_major(x, rows):  # [BL, H] slice -> [NB, H, NT] transposed tiles
        return np.ascontiguousarray(
            x[rows].reshape(NB, NT, H).transpose(0, 2, 1)
        ).astype(BF16)

    in_maps = []
    for i in range(NCORES):
        rows = slice(i * BL, (i + 1) * BL)
        in_maps.append({
            "hT": tile_major(h, rows),
            "eT": tile_major(e, rows),
            "w1T": w1T, "w2T": w2T, "pmT": pmT, "wrT": wrT,
            "consts": consts,
        })

    nc = _get_nc(lnb_zero)
    res = run_bass_kernel_spmd(nc, in_maps, core_ids=list(range(NCORES)),
                               trace=TRACE)
    LAST_RESULT["exec_time_ns"] = res.exec_time_ns
    LAST_RESULT["mean_exec_time_ns"] = res.mean_exec_time_ns
    LAST_RESULT["instructions_and_trace"] = res.instructions_and_trace

    out = np.empty((B, H), f)
    for i in range(NCORES):
        # [NB, H, NT] tile-major transposed -> [BL, H]
        out[i * BL : (i + 1) * BL] = (
            res.results[i]["out"].transpose(0, 2, 1).reshape(BL, H)
        )
    return out


# revision 63
# speedup vs baseline: 1.0581x; 1.0175x over previous
"""AdaptiveLiquidNeuron forward on 8 TRN2 NeuronCores (data-parallel over batch).

Math (per batch row, H=1024):
  context = relu(h @ W1.T + b1) @ W2.T + b2
  pa      = context @ PM.T + pm_b
  mm      = (1 + pa) * (e @ Wrec.T)
  dh      = (-decay*h + mm + bias) / (tau * sigmoid(pa))
  out     = LayerNorm(dh) * ln_w + ln_b

Strategy: shard B=16384 over 8 cores (2048 rows each), replicate H x H weights.
On-chip everything is kept transposed ([H on partitions, B on free]) so the four
matmuls need no on-chip transposes (host pre-transposes weights + activations).
LayerNorm reduces over the partition axis via ones-matmuls (sum and sum-of-
squares side by side in one rhs); rstd = 2*Dsqrt(var+eps) with the 2 folded into
ln_w host-side; stats broadcast back across partitions with one K=1 matmul.
Host folds 1/tau into Wrec/decay/bias, ce_b2 into pm_b, and uses
1/sigmoid(x) = 1 + exp(-x).
"""

import numpy as np
import ml_dtypes

BF16 = ml_dtypes.bfloat16

B, H = 16384, 1024
NCORES = 8
BL = B // NCORES      # 2048 batch rows per core
P = 128               # partitions
KC = H // P           # 8 chunks of the hidden dim
NB = 8                # batch tiles per core
NT = BL // NB         # 256 batch columns per tile
EPS = 1e-5

# consts layout: [128, 6*KC] f32, column v*KC + m = chunk m of vector v
V_B1, V_PMB, V_NDEC, V_BIASP, V_LNW2, V_LNB = range(6)

_CACHED = {}


def _build_nc(lnb_zero):
    import concourse.bass as bass
    import concourse.bacc as bacc
    import concourse.tile as tile
    from concourse import mybir
    from contextlib import ExitStack

    f32 = mybir.dt.float32
    bf16 = mybir.dt.bfloat16
    AF = mybir.ActivationFunctionType
    OP = mybir.AluOpType

    nc = bacc.Bacc(target_bir_lowering=False)

    # h/e/out are tile-major on DRAM ([NB, H, NT]) so every DMA is contiguous
    hT_e = nc.declare_dram_parameter("hT", [NB, H, NT], bf16, isOutput=False)
    eT_e = nc.declare_dram_parameter("eT", [NB, H, NT], bf16, isOutput=False)
    w1_e = nc.declare_dram_parameter("w1T", [H, H], bf16, isOutput=False)
    w2_e = nc.declare_dram_parameter("w2T", [H, H], bf16, isOutput=False)
    pm_e = nc.declare_dram_parameter("pmT", [H, H], bf16, isOutput=False)
    wr_e = nc.declare_dram_parameter("wrT", [H, H], bf16, isOutput=False)
    cs_e = nc.declare_dram_parameter("consts", [P, 6 * KC], f32, isOutput=False)
    out_e = nc.declare_dram_parameter("out", [NB, H, NT], f32, isOutput=True)

    hT_r = hT_e[:].rearrange("i (k p) b -> i p k b", p=P)
    eT_r = eT_e[:].rearrange("i (k p) b -> i p k b", p=P)
    out_r = out_e[:].rearrange("i (m p) b -> i p m b", p=P)

    with tile.TileContext(nc) as tc, ExitStack() as ctx:
        wpool = ctx.enter_context(tc.tile_pool(name="weights", bufs=1))
        cpool = ctx.enter_context(tc.tile_pool(name="consts", bufs=1))
        iopool = ctx.enter_context(tc.tile_pool(name="io", bufs=3))
        actpool = ctx.enter_context(tc.tile_pool(name="acts", bufs=1))
        fpool = ctx.enter_context(tc.tile_pool(name="f32work", bufs=1))
        dhpool = ctx.enter_context(tc.tile_pool(name="dh", bufs=2))
        rpool = ctx.enter_context(tc.tile_pool(name="redu", bufs=2))
        ypool = ctx.enter_context(tc.tile_pool(name="y", bufs=4))
        rowpool = ctx.enter_context(tc.tile_pool(name="rows", bufs=2))
        outpool = ctx.enter_context(tc.tile_pool(name="outs", bufs=1))
        bcpool = ctx.enter_context(tc.tile_pool(name="bc", bufs=2))
        drampool = ctx.enter_context(tc.tile_pool(name="dram", bufs=2,
                                                  space="DRAM"))
        psA = ctx.enter_context(tc.tile_pool(name="psA", bufs=5, space="PSUM"))
        psS = ctx.enter_context(tc.tile_pool(name="psS", bufs=1, space="PSUM"))

        # ---- resident constants / weights ----
        # Prologue latency: mm1 needs consts+w1+hT0 first — split those
        # halves across the sync and gpsimd DMA queues so they stream in
        # parallel; everything else queues up behind in need-order.
        consts = cpool.tile([P, 6 * KC], f32, tag="consts")
        nc.gpsimd.dma_start(out=consts[:], in_=cs_e[:])

        def col(v, m):
            return consts[:, v * KC + m : v * KC + m + 1]

        w_sb = {}
        for nm, ext in (("w1", w1_e), ("w2", w2_e), ("pm", pm_e), ("wr", wr_e)):
            w_sb[nm] = (wpool.tile([P, KC, H], bf16, tag=nm, name=f"w_{nm}"), ext)

        def load_w(nm, eng, lo=0, hi=KC):
            t, ext = w_sb[nm]
            src = ext[:].rearrange("(k p) m -> p k m", p=P)
            eng.dma_start(out=t[:, lo:hi, :], in_=src[:, lo:hi, :])
            return t

        def load_io(i, h_eng, e_eng, split=False):
            ht = iopool.tile([P, KC, NT], bf16, tag="hT")
            et = iopool.tile([P, KC, NT], bf16, tag="eT")
            if split:
                h_eng.dma_start(out=ht[:, 0 : KC // 2, :],
                                in_=hT_r[i, :, 0 : KC // 2, :])
                e_eng.dma_start(out=ht[:, KC // 2 :, :],
                                in_=hT_r[i, :, KC // 2 :, :])
            else:
                h_eng.dma_start(out=ht[:], in_=hT_r[i])
            e_eng.dma_start(out=et[:], in_=eT_r[i])
            return ht, et

        # Prologue: ~1MB pieces (descriptor-efficient) spread over the two
        # HWDGE rings (SP, ACT) + SWDGE (gpsimd) in the order compute needs
        # them: w1+h0 (mm1), w2 (mm2), pm (mm3), wr+e0 (mm4), then tile 1.
        ht0 = iopool.tile([P, KC, NT], bf16, tag="hT")
        et0 = iopool.tile([P, KC, NT], bf16, tag="eT")
        w1_sb = w_sb["w1"][0]
        for k in range(0, KC, 2):
            load_w("w1", nc.sync if k % 4 == 0 else nc.scalar, k, k + 2)
            (nc.scalar if k % 4 == 0 else nc.sync).dma_start(
                out=ht0[:, k : k + 2, :], in_=hT_r[0, :, k : k + 2, :]
            )
        w2_sb = load_w("w2", nc.sync, 0, 4)
        load_w("w2", nc.scalar, 4, 8)
        pm_sb = load_w("pm", nc.sync, 0, 4)
        load_w("pm", nc.scalar, 4, 8)
        wr_sb = load_w("wr", nc.sync, 0, 4)
        load_w("wr", nc.scalar, 4, 8)
        nc.gpsimd.dma_start(out=et0[:], in_=eT_r[0])
        io_tiles = [(ht0, et0), None]
        io_tiles[1] = load_io(1, nc.gpsimd, nc.gpsimd)

        ones_col = cpool.tile([P, 1], bf16, tag="ones_col")
        nc.vector.memset(ones_col[:], 1.0)
        ones_row = cpool.tile([1, P], f32, tag="ones_row")
        nc.vector.memset(ones_row[:], 1.0)

        # dummy matmuls during the prologue DMA wait: PE-HAM sees ~4us of
        # sustained activity and unthrottles to 2.4GHz before real work
        warm_w = cpool.tile([P, P], bf16, tag="warm_w")
        warm_x = cpool.tile([P, NT], bf16, tag="warm_x")
        nc.vector.memset(warm_w[:], 0.0)
        nc.vector.memset(warm_x[:], 0.0)
        warm_ps = psS.tile([1, 2 * NT], f32, tag="sums", name="warm_ps")
        for _ in range(40):
            nc.tensor.matmul(warm_ps[:, 0:NT], warm_w[:, 0:1], warm_x[:],
                             start=True, stop=True)


        state = [None] * NB

        def mm_layer(w, rhs_t, evac):
            """psum[m] = w[:,:,m].T @ rhs (contract KC chunks); evac(m, psum)."""
            for m in range(KC):
                acc = psA.tile([P, NT], f32, tag="acc")
                for k in range(KC):
                    nc.tensor.matmul(
                        acc[:],
                        w[:, k, m * P : (m + 1) * P],
                        rhs_t[:, k, :],
                        start=(k == 0),
                        stop=(k == KC - 1),
                    )
                evac(m, acc)

        def matmul_phase(i, pe_hook1, pe_hook2):
            ht, et = io_tiles[i % 2]
            if i + 2 < NB:
                io_tiles[i % 2] = load_io(i + 2, nc.sync, nc.sync)

            c1 = actpool.tile([P, KC, NT], bf16, tag="c1")
            cx = actpool.tile([P, KC, NT], bf16, tag="ctx")
            pa = fpool.tile([P, KC, NT], f32, tag="pa")
            ex = fpool.tile([P, KC, NT], f32, tag="exp")
            t2 = fpool.tile([P, KC, NT], f32, tag="t2")
            u = fpool.tile([P, KC, NT], f32, tag="u")
            num = fpool.tile([P, KC, NT], f32, tag="num")
            dh = dhpool.tile([P, KC, NT], f32, tag="dh")
            # dh (bf16) and dh^2 side by side so one ones-matmul per chunk
            # yields both sum and sum-of-squares
            red = rpool.tile([P, KC, 2 * NT], bf16, tag="red")

            # u only needs hT + consts: emit first so it clears the in-order
            # gpsimd queue before tile i-1's row math lands there
            for m in range(KC):
                nc.gpsimd.tensor_scalar(
                    u[:, m, :],
                    ht[:, m, :],
                    col(V_NDEC, m),
                    col(V_BIASP, m),
                    op0=OP.mult,
                    op1=OP.add,
                )

            # context encoder layer 1: c1 = relu(W1 @ hT + b1)
            def relu_evac(m, acc):
                nc.scalar.activation(
                    c1[:, m, :], acc[:], AF.Relu, bias=col(V_B1, m), scale=1.0
                )

            if i == 0:
                # k-outer in m-halves: consumes w1/hT chunks as the DMAs
                # land instead of waiting for the full tensors
                for half in range(2):
                    ms_ = range(half * 4, half * 4 + 4)
                    accs = [
                        psA.tile([P, NT], f32, tag="acc", name=f"acc0_{m}")
                        for m in ms_
                    ]
                    for k in range(KC):
                        for j, m in enumerate(ms_):
                            nc.tensor.matmul(
                                accs[j][:],
                                w1_sb[:, k, m * P : (m + 1) * P],
                                ht[:, k, :],
                                start=(k == 0),
                                stop=(k == KC - 1),
                            )
                    for j, m in enumerate(ms_):
                        relu_evac(m, accs[j])
            else:
                mm_layer(w1_sb, ht, relu_evac)
            pe_hook1()  # reductions of tile i-1 slot in here on PE
            # context encoder layer 2 (b2 folded into pm_b): ctx = W2 @ c1
            mm_layer(
                w2_sb,
                c1,
                lambda m, acc: nc.scalar.activation(
                    cx[:, m, :], acc[:], AF.Copy, bias=0.0, scale=1.0
                ),
            )
            pe_hook2()  # stat broadcast of tile i-1
            # param modulator: pa = PM @ ctx + pm_b'
            mm_layer(
                pm_sb,
                cx,
                lambda m, acc: nc.vector.tensor_scalar_add(
                    pa[:, m, :], acc[:], col(V_PMB, m)
                ),
            )
            # 1/sigmoid(pa) = 1 + exp(-pa)
            nc.scalar.activation(ex[:], pa[:], AF.Exp, bias=0.0, scale=-1.0)

            # recurrent: t2 = (1 + pa) * (Wrec' @ eT)
            mm_layer(
                wr_sb,
                et,
                lambda m, acc: nc.vector.scalar_tensor_tensor(
                    t2[:, m, :], pa[:, m, :], 1.0, acc[:], op0=OP.add, op1=OP.mult
                ),
            )

            # halves keep the tail latency down: reduce matmuls for half 0
            # can start while half 1 is still in the vector pipe
            nsplit = 4 if i == NB - 1 else 2  # short tail for the last tile
            step = KC // nsplit
            for s in [slice(j * step, (j + 1) * step) for j in range(nsplit)]:
                nc.vector.tensor_add(num[:, s, :], t2[:, s, :], u[:, s, :])
                # dh = num * (1 + exp(-pa))
                nc.vector.scalar_tensor_tensor(
                    dh[:, s, :], ex[:, s, :], 1.0, num[:, s, :],
                    op0=OP.add, op1=OP.mult,
                )
                nc.scalar.square(red[:, s, NT:], dh[:, s, :])
                nc.scalar.copy(red[:, s, 0:NT], dh[:, s, :])
            state[i] = (dh, red)

        def reduce_phase(i):
            # partition-axis sum+sumsq via ones-matmuls over all H=1024
            dh, red = state[i]
            s_ps = psS.tile([1, 2 * NT], f32, tag="sums")
            for m in range(KC):
                nc.tensor.matmul(
                    s_ps[:], ones_col[:], red[:, m, :],
                    start=(m == 0), stop=(m == KC - 1),
                )
            i32 = mybir.dt.int32
            mu_n = rowpool.tile([1, NT], f32, tag="mu_n")
            ms = rowpool.tile([1, NT], f32, tag="ms")
            musq = rowpool.tile([1, NT], f32, tag="musq")
            ve = rowpool.tile([1, NT], f32, tag="ve")
            yb = rowpool.tile([1, NT], f32, tag="yb")
            t1 = rowpool.tile([1, NT], f32, tag="t1")
            t2r = rowpool.tile([1, NT], f32, tag="t2r")
            dq = rowpool.tile([1, 2 * NT], f32, tag="dq")
            # rstd = rsqrt(var+eps) via Quake initial guess + one Newton
            # step (rel err ~2e-3, far below bf16 matmul noise) — avoids
            # ln/sqrt ACT funcs so the whole kernel stays in one
            # activation-table set (no table reloads)
            nc.vector.tensor_scalar_mul(mu_n[:], s_ps[:, 0:NT], -1.0 / H)
            nc.vector.tensor_scalar(ms[:], s_ps[:, NT:], 1.0 / H, EPS,
                                    op0=OP.mult, op1=OP.add)
            nc.vector.tensor_mul(musq[:], mu_n[:], mu_n[:])
            nc.vector.tensor_sub(ve[:], ms[:], musq[:])  # var + eps
            nc.vector.tensor_scalar(
                t1[:].bitcast(i32), ve[:].bitcast(i32), 1, None,
                op0=OP.arith_shift_right,
            )
            nc.vector.tensor_scalar(
                yb[:].bitcast(i32), t1[:].bitcast(i32), -1, 0x5F3759DF,
                op0=OP.mult, op1=OP.add,
            )
            # y1 = y0*(1.5 - 0.5*ve*y0^2)
            nc.vector.tensor_mul(t1[:], yb[:], yb[:])
            nc.vector.tensor_mul(t2r[:], t1[:], ve[:])
            nc.vector.tensor_scalar(t2r[:], t2r[:], -0.5, 1.5,
                                    op0=OP.mult, op1=OP.add)
            nc.vector.tensor_mul(dq[:, 0:NT], yb[:], t2r[:])
            nc.vector.tensor_mul(dq[:, NT:], mu_n[:], dq[:, 0:NT])
            state[i] = (dh, dq)

        def bcast_phase(i):
            # broadcast the [1, 2*NT] stats row across partitions via a DRAM
            # bounce + partition-step-0 read — costs no PE time. For the
            # last tile the PE is idle and DMA latency is the tail, so use a
            # K=1 ones-matmul there instead.
            dh, dq = state[i]
            if i == NB - 1:
                pq = psS.tile([P, 2 * NT], f32, tag="PQtail")
                nc.tensor.matmul(pq[:], ones_row[:], dq[:],
                                 start=True, stop=True)
            else:
                dqd = drampool.tile([2 * NT], f32, tag="dqd")
                nc.sync.dma_start(out=dqd[:], in_=dq[:])
                pq = bcpool.tile([P, 2 * NT], f32, tag="PQ")
                src = bass.AP(tensor=dqd.tensor, offset=dqd.offset,
                              ap=[[0, P]] + [list(a) for a in dqd.ap])
                nc.sync.dma_start(out=pq[:], in_=src)
            state[i] = (dh, pq)

        def epilogue_phase(i):
            dh, pq = state[i]
            outf = outpool.tile([P, KC, NT], f32, tag="outf")
            for m in range(KC):
                # out = lnw2*(dh*D + qn) (+ lnb) = ln_w*rstd*(dh-mu) + ln_b
                s1 = ypool.tile([P, NT], f32, tag="s1")
                nc.vector.scalar_tensor_tensor(
                    s1[:], dh[:, m, :], col(V_LNW2, m), pq[:, 0:NT],
                    op0=OP.mult, op1=OP.mult,
                )
                if lnb_zero:
                    nc.vector.scalar_tensor_tensor(
                        outf[:, m, :], pq[:, NT:], col(V_LNW2, m), s1[:],
                        op0=OP.mult, op1=OP.add,
                    )
                else:
                    s2 = ypool.tile([P, NT], f32, tag="s2")
                    nc.vector.scalar_tensor_tensor(
                        s2[:], pq[:, NT:], col(V_LNW2, m), s1[:],
                        op0=OP.mult, op1=OP.add,
                    )
                    nc.vector.tensor_scalar_add(outf[:, m, :], s2[:], col(V_LNB, m))
                if m % 2 == 1:  # stream results out as they complete
                    nc.sync.dma_start(
                        out=out_r[i, :, m - 1 : m + 1, :],
                        in_=outf[:, m - 1 : m + 1, :],
                    )
            state[i] = None

        for i in range(NB):
            matmul_phase(
                i,
                (lambda j=i: reduce_phase(j - 1)) if i > 0 else (lambda: None),
                (lambda j=i: bcast_phase(j - 1)) if i > 0 else (lambda: None),
            )
            if i > 0:
                epilogue_phase(i - 1)
        reduce_phase(NB - 1)
        bcast_phase(NB - 1)
        epilogue_phase(NB - 1)

    if not nc.is_finalized():
        nc.finalize()
    return nc


def _get_nc(lnb_zero):
    key = ("nc", lnb_zero)
    if key not in _CACHED:
        _CACHED[key] = _build_nc(lnb_zero)
    return _CACHED[key]


# test.py can flip these before calling kernel() to profile
TRACE = False
LAST_RESULT = {}


def kernel(t, h, e, W_rec, bias, tau, decay, ln_w, ln_b,
           ce_w1, ce_b1, ce_w2, ce_b2, pm_w, pm_b):
    from concourse.bass_utils import run_bass_kernel_spmd

    f = np.float32
    h = np.asarray(h, f)
    e = np.asarray(e, f)
    W_rec = np.asarray(W_rec, f)
    bias = np.asarray(bias, f)
    tau = np.asarray(tau, f)
    decay = np.asarray(decay, f)
    ln_w = np.asarray(ln_w, f)
    ln_b = np.asarray(ln_b, f)
    ce_w1 = np.asarray(ce_w1, f)
    ce_b1 = np.asarray(ce_b1, f)
    ce_w2 = np.asarray(ce_w2, f)
    ce_b2 = np.asarray(ce_b2, f)
    pm_w = np.asarray(pm_w, f)
    pm_b = np.asarray(pm_b, f)

    invtau = 1.0 / tau
    negdecay = -decay * invtau
    biasp = bias * invtau
    pmb_eff = pm_b + pm_w @ ce_b2  # fold ce_b2 through the param modulator
    lnb_zero = bool(np.all(ln_b == 0.0))

    w1T = np.ascontiguousarray(ce_w1.T).astype(BF16)
    w2T = np.ascontiguousarray(ce_w2.T).astype(BF16)
    pmT = np.ascontiguousarray(pm_w.T).astype(BF16)
    wrT = np.ascontiguousarray(W_rec.T * invtau[None, :]).astype(BF16)

    def chunked(v):  # [H] -> [128, KC] with column m = chunk m
        return np.ascontiguousarray(v.reshape(KC, P).T)

    consts = np.concatenate(
        [chunked(v) for v in (ce_b1, pmb_eff, negdecay, biasp, ln_w, ln_b)],
        axis=1,
    ).astype(f)

    def tile_major(x, rows):  # [BL, H] slice -> [NB, H, NT] transposed tiles
        return np.ascontiguousarray(
            x[rows].reshape(NB, NT, H).transpose(0, 2, 1)
        ).astype(BF16)

    in_maps = []
    for i in range(NCORES):
        rows = slice(i * BL, (i + 1) * BL)
        in_maps.append({
            "hT": tile_major(h, rows),
            "eT": tile_major(e, rows),
            "w1T": w1T, "w2T": w2T, "pmT": pmT, "wrT": wrT,
            "consts": consts,
        })

    nc = _get_nc(lnb_zero)
    res = run_bass_kernel_spmd(nc, in_maps, core_ids=list(range(NCORES)),
                               trace=TRACE)
    LAST_RESULT["exec_time_ns"] = res.exec_time_ns
    LAST_RESULT["mean_exec_time_ns"] = res.mean_exec_time_ns
    LAST_RESULT["instructions_and_trace"] = res.instructions_and_trace

    out = np.empty((B, H), f)
    for i in range(NCORES):
        # [NB, H, NT] tile-major transposed -> [BL, H]
        out[i * BL : (i + 1) * BL] = (
            res.results[i]["out"].transpose(0, 2, 1).reshape(BL, H)
        )
    return out


# revision 64
# speedup vs baseline: 1.0604x; 1.0022x over previous
"""AdaptiveLiquidNeuron forward on 8 TRN2 NeuronCores (data-parallel over batch).

Math (per batch row, H=1024):
  context = relu(h @ W1.T + b1) @ W2.T + b2
  pa      = context @ PM.T + pm_b
  mm      = (1 + pa) * (e @ Wrec.T)
  dh      = (-decay*h + mm + bias) / (tau * sigmoid(pa))
  out     = LayerNorm(dh) * ln_w + ln_b

Strategy: shard B=16384 over 8 cores (2048 rows each), replicate H x H weights.
On-chip everything is kept transposed ([H on partitions, B on free]) so the four
matmuls need no on-chip transposes (host pre-transposes weights + activations).
LayerNorm reduces over the partition axis via ones-matmuls (sum and sum-of-
squares side by side in one rhs); rstd = 2*Dsqrt(var+eps) with the 2 folded into
ln_w host-side; stats broadcast back across partitions with one K=1 matmul.
Host folds 1/tau into Wrec/decay/bias, ce_b2 into pm_b, and uses
1/sigmoid(x) = 1 + exp(-x).
"""

import numpy as np
import ml_dtypes

BF16 = ml_dtypes.bfloat16

B, H = 16384, 1024
NCORES = 8
BL = B // NCORES      # 2048 batch rows per core
P = 128               # partitions
KC = H // P           # 8 chunks of the hidden dim
NB = 8                # batch tiles per core
NT = BL // NB         # 256 batch columns per tile
EPS = 1e-5

# consts layout: [128, 6*KC] f32, column v*KC + m = chunk m of vector v
V_B1, V_PMB, V_NDEC, V_BIASP, V_LNW2, V_LNB = range(6)

_CACHED = {}


def _build_nc(lnb_zero):
    import concourse.bass as bass
    import concourse.bacc as bacc
    import concourse.tile as tile
    from concourse import mybir
    from contextlib import ExitStack

    f32 = mybir.dt.float32
    bf16 = mybir.dt.bfloat16
    AF = mybir.ActivationFunctionType
    OP = mybir.AluOpType

    nc = bacc.Bacc(target_bir_lowering=False)

    # h/e/out are tile-major on DRAM ([NB, H, NT]) so every DMA is contiguous
    hT_e = nc.declare_dram_parameter("hT", [NB, H, NT], bf16, isOutput=False)
    eT_e = nc.declare_dram_parameter("eT", [NB, H, NT], bf16, isOutput=False)
    w1_e = nc.declare_dram_parameter("w1T", [H, H], bf16, isOutput=False)
    w2_e = nc.declare_dram_parameter("w2T", [H, H], bf16, isOutput=False)
    pm_e = nc.declare_dram_parameter("pmT", [H, H], bf16, isOutput=False)
    wr_e = nc.declare_dram_parameter("wrT", [H, H], bf16, isOutput=False)
    cs_e = nc.declare_dram_parameter("consts", [P, 6 * KC], f32, isOutput=False)
    out_e = nc.declare_dram_parameter("out", [NB, H, NT], f32, isOutput=True)

    hT_r = hT_e[:].rearrange("i (k p) b -> i p k b", p=P)
    eT_r = eT_e[:].rearrange("i (k p) b -> i p k b", p=P)
    out_r = out_e[:].rearrange("i (m p) b -> i p m b", p=P)

    with tile.TileContext(nc) as tc, ExitStack() as ctx:
        wpool = ctx.enter_context(tc.tile_pool(name="weights", bufs=1))
        cpool = ctx.enter_context(tc.tile_pool(name="consts", bufs=1))
        iopool = ctx.enter_context(tc.tile_pool(name="io", bufs=3))
        actpool = ctx.enter_context(tc.tile_pool(name="acts", bufs=1))
        fpool = ctx.enter_context(tc.tile_pool(name="f32work", bufs=1))
        dhpool = ctx.enter_context(tc.tile_pool(name="dh", bufs=2))
        rpool = ctx.enter_context(tc.tile_pool(name="redu", bufs=2))
        ypool = ctx.enter_context(tc.tile_pool(name="y", bufs=4))
        rowpool = ctx.enter_context(tc.tile_pool(name="rows", bufs=2))
        outpool = ctx.enter_context(tc.tile_pool(name="outs", bufs=1))
        bcpool = ctx.enter_context(tc.tile_pool(name="bc", bufs=2))
        drampool = ctx.enter_context(tc.tile_pool(name="dram", bufs=2,
                                                  space="DRAM"))
        psA = ctx.enter_context(tc.tile_pool(name="psA", bufs=6, space="PSUM"))
        psS = ctx.enter_context(tc.tile_pool(name="psS", bufs=1, space="PSUM"))

        # ---- resident constants / weights ----
        # Prologue latency: mm1 needs consts+w1+hT0 first — split those
        # halves across the sync and gpsimd DMA queues so they stream in
        # parallel; everything else queues up behind in need-order.
        consts = cpool.tile([P, 6 * KC], f32, tag="consts")
        nc.gpsimd.dma_start(out=consts[:], in_=cs_e[:])

        def col(v, m):
            return consts[:, v * KC + m : v * KC + m + 1]

        w_sb = {}
        for nm, ext in (("w1", w1_e), ("w2", w2_e), ("pm", pm_e), ("wr", wr_e)):
            w_sb[nm] = (wpool.tile([P, KC, H], bf16, tag=nm, name=f"w_{nm}"), ext)

        def load_w(nm, eng, lo=0, hi=KC):
            t, ext = w_sb[nm]
            src = ext[:].rearrange("(k p) m -> p k m", p=P)
            eng.dma_start(out=t[:, lo:hi, :], in_=src[:, lo:hi, :])
            return t

        def load_io(i, h_eng, e_eng, split=False):
            ht = iopool.tile([P, KC, NT], bf16, tag="hT")
            et = iopool.tile([P, KC, NT], bf16, tag="eT")
            if split:
                h_eng.dma_start(out=ht[:, 0 : KC // 2, :],
                                in_=hT_r[i, :, 0 : KC // 2, :])
                e_eng.dma_start(out=ht[:, KC // 2 :, :],
                                in_=hT_r[i, :, KC // 2 :, :])
            else:
                h_eng.dma_start(out=ht[:], in_=hT_r[i])
            e_eng.dma_start(out=et[:], in_=eT_r[i])
            return ht, et

        # Prologue: ~1MB pieces (descriptor-efficient) spread over the two
        # HWDGE rings (SP, ACT) + SWDGE (gpsimd) in the order compute needs
        # them: w1+h0 (mm1), w2 (mm2), pm (mm3), wr+e0 (mm4), then tile 1.
        ht0 = iopool.tile([P, KC, NT], bf16, tag="hT")
        et0 = iopool.tile([P, KC, NT], bf16, tag="eT")
        w1_sb = w_sb["w1"][0]
        for k in range(0, KC, 2):
            load_w("w1", nc.sync if k % 4 == 0 else nc.scalar, k, k + 2)
            (nc.scalar if k % 4 == 0 else nc.sync).dma_start(
                out=ht0[:, k : k + 2, :], in_=hT_r[0, :, k : k + 2, :]
            )
        w2_sb = load_w("w2", nc.sync, 0, 4)
        load_w("w2", nc.scalar, 4, 8)
        pm_sb = load_w("pm", nc.sync, 0, 4)
        load_w("pm", nc.scalar, 4, 8)
        wr_sb = load_w("wr", nc.sync, 0, 4)
        load_w("wr", nc.scalar, 4, 8)
        nc.gpsimd.dma_start(out=et0[:], in_=eT_r[0])
        io_tiles = [(ht0, et0), None]
        io_tiles[1] = load_io(1, nc.gpsimd, nc.gpsimd)

        ones_col = cpool.tile([P, 1], bf16, tag="ones_col")
        nc.vector.memset(ones_col[:], 1.0)
        ones_row = cpool.tile([1, P], f32, tag="ones_row")
        nc.vector.memset(ones_row[:], 1.0)

        # dummy matmuls during the prologue DMA wait: PE-HAM sees ~4us of
        # sustained activity and unthrottles to 2.4GHz before real work
        warm_w = cpool.tile([P, P], bf16, tag="warm_w")
        warm_x = cpool.tile([P, NT], bf16, tag="warm_x")
        nc.vector.memset(warm_w[:], 0.0)
        nc.vector.memset(warm_x[:], 0.0)
        warm_ps = psS.tile([1, 2 * NT], f32, tag="sums", name="warm_ps")
        for _ in range(40):
            nc.tensor.matmul(warm_ps[:, 0:NT], warm_w[:, 0:1], warm_x[:],
                             start=True, stop=True)


        state = [None] * NB

        def mm_layer(w, rhs_t, evac):
            """psum[m] = w[:,:,m].T @ rhs (contract KC chunks); evac(m, psum)."""
            for m in range(KC):
                acc = psA.tile([P, NT], f32, tag="acc")
                for k in range(KC):
                    nc.tensor.matmul(
                        acc[:],
                        w[:, k, m * P : (m + 1) * P],
                        rhs_t[:, k, :],
                        start=(k == 0),
                        stop=(k == KC - 1),
                    )
                evac(m, acc)

        def matmul_phase(i, pe_hook1, pe_hook2):
            ht, et = io_tiles[i % 2]
            if i + 2 < NB:
                io_tiles[i % 2] = load_io(i + 2, nc.sync, nc.sync)

            c1 = actpool.tile([P, KC, NT], bf16, tag="c1")
            cx = actpool.tile([P, KC, NT], bf16, tag="ctx")
            pa = fpool.tile([P, KC, NT], f32, tag="pa")
            ex = fpool.tile([P, KC, NT], f32, tag="exp")
            t2 = fpool.tile([P, KC, NT], f32, tag="t2")
            u = fpool.tile([P, KC, NT], f32, tag="u")
            num = fpool.tile([P, KC, NT], f32, tag="num")
            dh = dhpool.tile([P, KC, NT], f32, tag="dh")
            # dh (bf16) and dh^2 side by side so one ones-matmul per chunk
            # yields both sum and sum-of-squares
            red = rpool.tile([P, KC, 2 * NT], bf16, tag="red")

            # u only needs hT + consts: emit first so it clears the in-order
            # gpsimd queue before tile i-1's row math lands there
            for m in range(KC):
                nc.gpsimd.tensor_scalar(
                    u[:, m, :],
                    ht[:, m, :],
                    col(V_NDEC, m),
                    col(V_BIASP, m),
                    op0=OP.mult,
                    op1=OP.add,
                )

            # context encoder layer 1: c1 = relu(W1 @ hT + b1)
            def relu_evac(m, acc):
                nc.scalar.activation(
                    c1[:, m, :], acc[:], AF.Relu, bias=col(V_B1, m), scale=1.0
                )

            if i == 0:
                # k-outer in m-halves: consumes w1/hT chunks as the DMAs
                # land instead of waiting for the full tensors
                for half in range(2):
                    ms_ = range(half * 4, half * 4 + 4)
                    accs = [
                        psA.tile([P, NT], f32, tag="acc", name=f"acc0_{m}")
                        for m in ms_
                    ]
                    for k in range(KC):
                        for j, m in enumerate(ms_):
                            nc.tensor.matmul(
                                accs[j][:],
                                w1_sb[:, k, m * P : (m + 1) * P],
                                ht[:, k, :],
                                start=(k == 0),
                                stop=(k == KC - 1),
                            )
                    for j, m in enumerate(ms_):
                        relu_evac(m, accs[j])
            else:
                mm_layer(w1_sb, ht, relu_evac)
            pe_hook1()  # reductions of tile i-1 slot in here on PE
            # context encoder layer 2 (b2 folded into pm_b): ctx = W2 @ c1
            mm_layer(
                w2_sb,
                c1,
                lambda m, acc: nc.scalar.activation(
                    cx[:, m, :], acc[:], AF.Copy, bias=0.0, scale=1.0
                ),
            )
            pe_hook2()  # stat broadcast of tile i-1
            # param modulator: pa = PM @ ctx + pm_b'
            mm_layer(
                pm_sb,
                cx,
                lambda m, acc: nc.vector.tensor_scalar_add(
                    pa[:, m, :], acc[:], col(V_PMB, m)
                ),
            )
            # 1/sigmoid(pa) = 1 + exp(-pa)
            nc.scalar.activation(ex[:], pa[:], AF.Exp, bias=0.0, scale=-1.0)

            # recurrent: t2 = (1 + pa) * (Wrec' @ eT)
            mm_layer(
                wr_sb,
                et,
                lambda m, acc: nc.vector.scalar_tensor_tensor(
                    t2[:, m, :], pa[:, m, :], 1.0, acc[:], op0=OP.add, op1=OP.mult
                ),
            )

            # halves keep the tail latency down: reduce matmuls for half 0
            # can start while half 1 is still in the vector pipe
            nsplit = 4 if i == NB - 1 else 2  # short tail for the last tile
            step = KC // nsplit
            for s in [slice(j * step, (j + 1) * step) for j in range(nsplit)]:
                nc.vector.tensor_add(num[:, s, :], t2[:, s, :], u[:, s, :])
                # dh = num * (1 + exp(-pa))
                nc.vector.scalar_tensor_tensor(
                    dh[:, s, :], ex[:, s, :], 1.0, num[:, s, :],
                    op0=OP.add, op1=OP.mult,
                )
                nc.scalar.square(red[:, s, NT:], dh[:, s, :])
                nc.scalar.copy(red[:, s, 0:NT], dh[:, s, :])
            state[i] = (dh, red)

        def reduce_phase(i):
            # partition-axis sum+sumsq via ones-matmuls over all H=1024
            dh, red = state[i]
            s_ps = psS.tile([1, 2 * NT], f32, tag="sums")
            for m in range(KC):
                nc.tensor.matmul(
                    s_ps[:], ones_col[:], red[:, m, :],
                    start=(m == 0), stop=(m == KC - 1),
                )
            i32 = mybir.dt.int32
            mu_n = rowpool.tile([1, NT], f32, tag="mu_n")
            ms = rowpool.tile([1, NT], f32, tag="ms")
            musq = rowpool.tile([1, NT], f32, tag="musq")
            ve = rowpool.tile([1, NT], f32, tag="ve")
            yb = rowpool.tile([1, NT], f32, tag="yb")
            t1 = rowpool.tile([1, NT], f32, tag="t1")
            t2r = rowpool.tile([1, NT], f32, tag="t2r")
            dq = rowpool.tile([1, 2 * NT], f32, tag="dq")
            # rstd = rsqrt(var+eps) via Quake initial guess + one Newton
            # step (rel err ~2e-3, far below bf16 matmul noise) — avoids
            # ln/sqrt ACT funcs so the whole kernel stays in one
            # activation-table set (no table reloads)
            nc.vector.tensor_scalar_mul(mu_n[:], s_ps[:, 0:NT], -1.0 / H)
            nc.vector.tensor_scalar(ms[:], s_ps[:, NT:], 1.0 / H, EPS,
                                    op0=OP.mult, op1=OP.add)
            nc.vector.tensor_mul(musq[:], mu_n[:], mu_n[:])
            nc.vector.tensor_sub(ve[:], ms[:], musq[:])  # var + eps
            nc.vector.tensor_scalar(
                t1[:].bitcast(i32), ve[:].bitcast(i32), 1, None,
                op0=OP.arith_shift_right,
            )
            nc.vector.tensor_scalar(
                yb[:].bitcast(i32), t1[:].bitcast(i32), -1, 0x5F3759DF,
                op0=OP.mult, op1=OP.add,
            )
            # y1 = y0*(1.5 - 0.5*ve*y0^2)
            nc.vector.tensor_mul(t1[:], yb[:], yb[:])
            nc.vector.tensor_mul(t2r[:], t1[:], ve[:])
            nc.vector.tensor_scalar(t2r[:], t2r[:], -0.5, 1.5,
                                    op0=OP.mult, op1=OP.add)
            nc.vector.tensor_mul(dq[:, 0:NT], yb[:], t2r[:])
            nc.vector.tensor_mul(dq[:, NT:], mu_n[:], dq[:, 0:NT])
            state[i] = (dh, dq)

        def bcast_phase(i):
            # broadcast the [1, 2*NT] stats row across partitions via a DRAM
            # bounce + partition-step-0 read — costs no PE time. For the
            # last tile the PE is idle and DMA latency is the tail, so use a
            # K=1 ones-matmul there instead.
            dh, dq = state[i]
            if i == NB - 1:
                pq = psS.tile([P, 2 * NT], f32, tag="PQtail")
                nc.tensor.matmul(pq[:], ones_row[:], dq[:],
                                 start=True, stop=True)
            else:
                dqd = drampool.tile([2 * NT], f32, tag="dqd")
                nc.sync.dma_start(out=dqd[:], in_=dq[:])
                pq = bcpool.tile([P, 2 * NT], f32, tag="PQ")
                src = bass.AP(tensor=dqd.tensor, offset=dqd.offset,
                              ap=[[0, P]] + [list(a) for a in dqd.ap])
                nc.sync.dma_start(out=pq[:], in_=src)
            state[i] = (dh, pq)

        def epilogue_phase(i):
            dh, pq = state[i]
            outf = outpool.tile([P, KC, NT], f32, tag="outf")
            for m in range(KC):
                # out = lnw2*(dh*D + qn) (+ lnb) = ln_w*rstd*(dh-mu) + ln_b
                s1 = ypool.tile([P, NT], f32, tag="s1")
                nc.vector.scalar_tensor_tensor(
                    s1[:], dh[:, m, :], col(V_LNW2, m), pq[:, 0:NT],
                    op0=OP.mult, op1=OP.mult,
                )
                if lnb_zero:
                    nc.vector.scalar_tensor_tensor(
                        outf[:, m, :], pq[:, NT:], col(V_LNW2, m), s1[:],
                        op0=OP.mult, op1=OP.add,
                    )
                else:
                    s2 = ypool.tile([P, NT], f32, tag="s2")
                    nc.vector.scalar_tensor_tensor(
                        s2[:], pq[:, NT:], col(V_LNW2, m), s1[:],
                        op0=OP.mult, op1=OP.add,
                    )
                    nc.vector.tensor_scalar_add(outf[:, m, :], s2[:], col(V_LNB, m))
                if m % 2 == 1:  # stream results out as they complete
                    nc.sync.dma_start(
                        out=out_r[i, :, m - 1 : m + 1, :],
                        in_=outf[:, m - 1 : m + 1, :],
                    )
            state[i] = None

        for i in range(NB):
            matmul_phase(
                i,
                (lambda j=i: reduce_phase(j - 1)) if i > 0 else (lambda: None),
                (lambda j=i: bcast_phase(j - 1)) if i > 0 else (lambda: None),
            )
            if i > 0:
                epilogue_phase(i - 1)
        reduce_phase(NB - 1)
        bcast_phase(NB - 1)
        epilogue_phase(NB - 1)

    if not nc.is_finalized():
        nc.finalize()
    return nc


def _get_nc(lnb_zero):
    key = ("nc", lnb_zero)
    if key not in _CACHED:
        _CACHED[key] = _build_nc(lnb_zero)
    return _CACHED[key]


# test.py can flip these before calling kernel() to profile
TRACE = False
LAST_RESULT = {}


def kernel(t, h, e, W_rec, bias, tau, decay, ln_w, ln_b,
           ce_w1, ce_b1, ce_w2, ce_b2, pm_w, pm_b):
    from concourse.bass_utils import run_bass_kernel_spmd

    f = np.float32
    h = np.asarray(h, f)
    e = np.asarray(e, f)
    W_rec = np.asarray(W_rec, f)
    bias = np.asarray(bias, f)
    tau = np.asarray(tau, f)
    decay = np.asarray(decay, f)
    ln_w = np.asarray(ln_w, f)
    ln_b = np.asarray(ln_b, f)
    ce_w1 = np.asarray(ce_w1, f)
    ce_b1 = np.asarray(ce_b1, f)
    ce_w2 = np.asarray(ce_w2, f)
    ce_b2 = np.asarray(ce_b2, f)
    pm_w = np.asarray(pm_w, f)
    pm_b = np.asarray(pm_b, f)

    invtau = 1.0 / tau
    negdecay = -decay * invtau
    biasp = bias * invtau
    pmb_eff = pm_b + pm_w @ ce_b2  # fold ce_b2 through the param modulator
    lnb_zero = bool(np.all(ln_b == 0.0))

    w1T = np.ascontiguousarray(ce_w1.T).astype(BF16)
    w2T = np.ascontiguousarray(ce_w2.T).astype(BF16)
    pmT = np.ascontiguousarray(pm_w.T).astype(BF16)
    wrT = np.ascontiguousarray(W_rec.T * invtau[None, :]).astype(BF16)

    def chunked(v):  # [H] -> [128, KC] with column m = chunk m
        return np.ascontiguousarray(v.reshape(KC, P).T)

    consts = np.concatenate(
        [chunked(v) for v in (ce_b1, pmb_eff, negdecay, biasp, ln_w, ln_b)],
        axis=1,
    ).astype(f)

    def tile_major(x, rows):  # [BL, H] slice -> [NB, H, NT] transposed tiles
        return np.ascontiguousarray(
            x[rows].reshape(NB, NT, H).transpose(0, 2, 1)
        ).astype(BF16)

    in_maps = []
    for i in range(NCORES):
        rows = slice(i * BL, (i + 1) * BL)
        in_maps.append({
            "hT": tile_major(h, rows),
            "eT": tile_major(e, rows),
            "w1T": w1T, "w2T": w2T, "pmT": pmT, "wrT": wrT,
            "consts": consts,
        })

    nc = _get_nc(lnb_zero)
    res = run_bass_kernel_spmd(nc, in_maps, core_ids=list(range(NCORES)),
                               trace=TRACE)
    LAST_RESULT["exec_time_ns"] = res.exec_time_ns
    LAST_RESULT["mean_exec_time_ns"] = res.mean_exec_time_ns
    LAST_RESULT["instructions_and_trace"] = res.instructions_and_trace

    out = np.empty((B, H), f)
    for i in range(NCORES):
        # [NB, H, NT] tile-major transposed -> [BL, H]
        out[i * BL : (i + 1) * BL] = (
            res.results[i]["out"].transpose(0, 2, 1).reshape(BL, H)
        )
    return out
